# revision 25
# baseline (speedup 1.0000x reference)
"""GATv2 3-layer GNN on 8 Trainium2 NeuronCores.

Strategy (dst-sharded edge processing, single gather per edge):
- Nodes are bin-packed by in-degree into 8*NB bins of <=128 dst nodes each,
  balancing edges per bin. Bin -> (core, block). The xl feature table uses a
  CHUNK-major global layout (chunk, core, block, row) so the AllGather can be
  split into G contiguous chunks interleaved with compute.
- Per layer, per core: xl/xr for own nodes via PE (transpose + one [Wl|Wr]
  matmul per block); xl shards AllGather'ed chunk-by-chunk (bf16); xr kept
  in SBUF. The next layer's node transforms and AllGather chunks are emitted
  INSIDE the previous layer's edge loop so they overlap edge processing.
- Edges (grouped by dst block, padded to uniform tile counts) are processed
  in super-chunks of C blocks: one dma_gather of xl rows per edge (HBM,
  edge-major); xr per edge comes from a transposed SBUF-source dma_gather
  (channel-major, straight from xr_pin); z_pre = xl^T + xr via two
  identity-weight matmuls into PSUM; leaky-relu on ScalarE; logits via PE
  (ldweights = z^T trick); exp on ScalarE (duplicated pairs for DVE 2x);
  messages w*xl on VectorE (2x mode); segment-sum via one-hot matmuls into
  PSUM (one-hots built in ONE DVE op per super-chunk with a pair-split AP).
- Segment softmax skips max-subtraction (logits are O(1) by construction).
- int16 gather indices: edges are split per block into a "lo" group
  (table rows < 32768, base 0) and "hi" group (rows >= BBASE, base BBASE).
"""
import math
import numpy as np
import ml_dtypes

import concourse.bacc as bacc
import concourse.bass as bass
import concourse.mybir as mybir
import concourse.tile as tile
from concourse.library_config import mlp as mlp_lib


# --- patch Tile's DMASW lane assignment to be SWDGE-queue-aware: a DMA sem
# lane must only ever be updated from one SWDGE queue; Tile round-robins
# lanes obliviously. Pin lanes {2q, 2q+1} to queue q (NQ<=4).
from concourse import tile_sem_assignment as _tsa


def _queue_aware_assign_tick(self, inst, *, _orig=_tsa.TileClockTick._assign_tick):
    if (isinstance(inst, _tsa.DMAInst)
            and inst.engine == mybir.EngineType.Pool):
        q = int(getattr(inst, "queue_num", 0) or 0)
        if not hasattr(self, "_qtog"):
            self._qtog = {}
        t = self._qtog.get(q, 0)
        lanes = max(1, self.swdge_sem_count // 4)
        self.next_sw_dma_idx = (q * lanes + t) % self.swdge_sem_count
        self._qtog[q] = (t + 1) % lanes
    return _orig(self, inst)


_tsa.TileClockTick._assign_tick = _queue_aware_assign_tick

F32 = mybir.dt.float32
BF16 = mybir.dt.bfloat16
I16 = mybir.dt.int16
BF = ml_dtypes.bfloat16
AF = mybir.ActivationFunctionType
OP = mybir.AluOpType

NCORES = 8
IN, HID, H, OUT, NLAYERS = 128, 32, 4, 1, 3
FH = HID * H  # 128
L = NLAYERS


class Cfg:
    def __init__(self, N, E_raw, NB, C=2, SPLIT=32768, NQ=4, G=5):
        self.N = N
        self.NB = NB            # blocks (bins) per core
        self.C = C              # blocks per super-chunk
        assert NB % C == 0
        self.NSC = NB // C
        self.NPC = NB * 128     # table rows per core
        self.NTAB = NCORES * self.NPC
        self.SPLIT = min(SPLIT, self.NTAB)
        self.BBASE = max(0, self.NTAB - self.SPLIT)  # hi-group table base
        self.NQ = NQ
        self.G = G              # AllGather chunks per layer
        assert NB % G == 0
        self.BPC = NB // G      # blocks per AG chunk (per core)
        assert self.NSC % G == 0
        self.SCPC = self.NSC // G   # super-chunks per AG chunk
        assert self.NTAB - self.BBASE <= self.SPLIT


FULL = Cfg(N=50000, E_raw=800000, NB=50, G=1)
DEBUG = False


# ---------------------------------------------------------------- host side


def _wrap_idx16(idx, pad_to):
    """[n] ints -> [128, pad_to//16] int16 dma_gather index layout
    (i at partition i%16, col i//16; replicated into all 8 groups)."""
    a = np.zeros(pad_to, np.int64)
    a[: len(idx)] = idx
    w = a.reshape(pad_to // 16, 16).T.astype(np.int16)
    return np.tile(w, (8, 1))


def table_row(cfg, core, bidx, r):
    """Chunk-major table layout: (chunk, core, block-in-chunk, row)."""
    ch = bidx // cfg.BPC
    return (ch * NCORES * cfg.BPC * 128 + core * cfg.BPC * 128
            + (bidx % cfg.BPC) * 128 + r)


def preprocess(cfg, edge_index):
    N, NB, C = cfg.N, cfg.NB, cfg.C
    nbins = NCORES * NB
    src = np.concatenate([np.asarray(edge_index[0], np.int64),
                          np.arange(N, dtype=np.int64)])
    dst = np.concatenate([np.asarray(edge_index[1], np.int64),
                          np.arange(N, dtype=np.int64)])
    deg = np.bincount(dst, minlength=N)

    # snake-deal nodes (sorted by in-degree desc) into bins
    order = np.argsort(-deg, kind="stable")
    nrounds = math.ceil(N / nbins)
    binof = np.empty(N, np.int64)
    rowof = np.empty(N, np.int64)
    for r in range(nrounds):
        chunk = order[r * nbins:(r + 1) * nbins]
        cols = np.arange(len(chunk))
        if r % 2:
            cols = nbins - 1 - cols
        binof[chunk] = cols
        rowof[chunk] = r
    assert rowof.max() <= 127
    coreof = binof // NB
    bidxof = binof % NB
    perm_pos = table_row(cfg, coreof, bidxof, rowof)   # node -> table row

    psrc = perm_pos[src]
    ebin = binof[dst]
    erow = rowof[dst]

    eorder = np.argsort(ebin, kind="stable")
    psrc = psrc[eorder]
    erow = erow[eorder]
    counts = np.bincount(ebin[eorder], minlength=nbins)
    starts = np.concatenate([[0], np.cumsum(counts)])

    # lo/hi split with flexible band
    grp = np.zeros(len(psrc), np.int8)
    nlo = np.zeros(nbins, np.int64)
    for b in range(nbins):
        s, e = starts[b], starts[b + 1]
        p = psrc[s:e]
        lofix = p < cfg.BBASE
        hifix = p >= cfg.SPLIT
        flex = ~lofix & ~hifix
        a, bb, f = int(lofix.sum()), int(hifix.sum()), int(flex.sum())
        x = int(np.clip((bb + f - a + 1) // 2, 0, f))
        g = np.zeros(e - s, np.int8)
        g[hifix] = 1
        fi = np.nonzero(flex)[0]
        g[fi[x:]] = 1
        grp[s:e] = g
        nlo[b] = a + x
    nhi = counts - nlo
    if cfg.NTAB <= cfg.SPLIT:
        grp[:] = 0
        nlo = counts.copy()
        nhi[:] = 0
    TLO = max(1, int(np.ceil(nlo.max() / 128)))
    THI = int(np.ceil(nhi.max() / 128))
    TT = TLO + THI
    CT = C * TT

    idx_lo = np.zeros((NCORES, cfg.NSC, 128, C * TLO * 128 // 16), np.int16)
    idx_hi = np.zeros((NCORES, cfg.NSC, 128, max(1, C * THI * 128 // 16)),
                      np.int16)
    idx_xr = np.zeros((NCORES, cfg.NSC, 128, CT * 128 // 16), np.int16)
    dstloc = np.full((NCORES, cfg.NSC, CT, 128), 255.0, np.float32)

    for core in range(NCORES):
        for sc in range(cfg.NSC):
            blocks = [core * NB + sc * C + j for j in range(C)]
            lo_list, hi_list = [], []
            for j, b in enumerate(blocks):
                s, e = starts[b], starts[b + 1]
                g = grp[s:e]
                p = psrc[s:e].copy()
                r = erow[s:e]
                for gi, (tbase, tcnt, lst) in enumerate(
                    ((0, TLO, lo_list), (C * TLO, THI, hi_list))
                ):
                    sel = g == gi
                    pp = p[sel]
                    rr = r[sel]
                    if gi == 1:
                        pp = pp - cfg.BBASE
                    assert len(pp) <= tcnt * 128
                    pad = tcnt * 128 - len(pp)
                    ppad = np.concatenate([pp, np.zeros(pad, np.int64)])
                    dpad = np.concatenate(
                        [rr.astype(np.float32),
                         np.full(pad, 255.0, np.float32)])
                    lst.append(ppad)
                    for t in range(tcnt):
                        gt = tbase + j * tcnt + t
                        dstloc[core, sc, gt] = dpad[t * 128:(t + 1) * 128]
            idx_lo[core, sc] = _wrap_idx16(np.concatenate(lo_list),
                                           C * TLO * 128)
            if THI:
                idx_hi[core, sc] = _wrap_idx16(np.concatenate(hi_list),
                                               C * THI * 128)
            # xr gather indices: local node idx (block_local*128 + dstrow);
            # pads (255) -> 0
            dloc = dstloc[core, sc]                      # [CT, 128]
            jof = np.empty(CT, np.int64)
            jof[:C * TLO] = np.repeat(np.arange(C), TLO)
            if THI:
                jof[C * TLO:] = np.repeat(np.arange(C), THI)
            lidx = ((sc * C + jof)[:, None] * 128
                    + np.where(dloc < 255.0, dloc, 0.0).astype(np.int64))
            lidx[dloc >= 255.0] = 0
            idx_xr[core, sc] = _wrap_idx16(lidx.reshape(-1), CT * 128)

    # dstloc -> [128 edge-row, CT] per (core, sc)
    dstloc_t = np.ascontiguousarray(dstloc.transpose(0, 1, 3, 2))
    return dict(perm_pos=perm_pos, TLO=TLO, THI=THI,
                idx_lo=idx_lo, idx_hi=idx_hi, idx_xr=idx_xr,
                dstloc=dstloc_t.astype(BF))


def make_ablk(att_l):  # [H, HID] -> [FH, H]
    a = np.zeros((FH, H), np.float32)
    for h in range(H):
        a[h * HID:(h + 1) * HID, h] = att_l[h]
    return a


# ---------------------------------------------------------------- program


def build_program(cfg, TLO, THI):
    nc = bacc.Bacc("TRN2", target_bir_lowering=False, debug=False,
                   num_devices=NCORES, num_swdge_queues=cfg.NQ)
    NB, C, NSC, NPC, NTAB = cfg.NB, cfg.C, cfg.NSC, cfg.NPC, cfg.NTAB
    G, BPC, SCPC = cfg.G, cfg.BPC, cfg.SCPC
    TT = TLO + THI
    CT = C * TT

    x_in = nc.dram_tensor("xp", [128, NPC], BF16, kind="ExternalInput")
    ilo = nc.dram_tensor("ilo", [NSC, 128, C * TLO * 128 // 16], I16,
                         kind="ExternalInput")
    ihi = nc.dram_tensor("ihi", [NSC, 128, max(1, C * THI * 128 // 16)], I16,
                         kind="ExternalInput")
    ixr = nc.dram_tensor("ixr", [NSC, 128, CT * 128 // 16], I16,
                         kind="ExternalInput")
    dstl = nc.dram_tensor("dstl", [NSC, 128, CT], BF16, kind="ExternalInput")
    wlr_in = nc.dram_tensor("wlr", [FH, L * 2 * FH], BF16,
                            kind="ExternalInput")
    ab_in = nc.dram_tensor("ab", [FH, L * H], BF16, kind="ExternalInput")
    blr_in = nc.dram_tensor("blr", [1, L * 2 * FH], F32, kind="ExternalInput")
    bo_in = nc.dram_tensor("bo", [128, L * FH], F32, kind="ExternalInput")
    wf_in = nc.dram_tensor("wfb", [128, FH], F32, kind="ExternalInput")
    bf_in = nc.dram_tensor("bfb", [128, 1], F32, kind="ExternalInput")
    iota_in = nc.dram_tensor("iota", [128, 128], BF16, kind="ExternalInput")
    id_in = nc.dram_tensor("ident", [128, 128], BF16, kind="ExternalInput")
    out_t = nc.dram_tensor("out", [128, NB], F32, kind="ExternalOutput")
    dbg = {}
    if DEBUG:
        for nm, w, dt_ in (("xln", CT * 128, BF16), ("xrn", CT * 128, BF16),
                           ("zT", CT * 128, BF16), ("lgs2", CT * H * 2, BF16),
                           ("mw", CT * 128, BF16), ("acc", C * 128, F32),
                           ("den", C * 4, F32), ("hn", C * 128, BF16),
                           ("oh", CT * 128, BF16), ("xls", 2 * 128, BF16),
                           ("xrp", 2 * 128, BF16)):
            dbg[nm] = nc.dram_tensor(f"dbg_{nm}", [128, w], dt_,
                                     kind="ExternalOutput")
        dbg["xlf"] = nc.dram_tensor("dbg_xlf", [NTAB, FH], BF16,
                                    kind="ExternalOutput")
        for li in range(L):
            dbg[f"h{li}"] = nc.dram_tensor(f"dbg_h{li}", [128, NPC],
                                           BF16 if li < L - 1 else F32,
                                           kind="ExternalOutput")

    TG = 4                      # z tiles per PSUM group
    NZG = math.ceil(CT / TG)

    with tile.TileContext(nc) as tc:
        with (
            tc.tile_pool(name="const", bufs=1) as cpool,
            tc.tile_pool(name="big", bufs=1) as bigp,
            tc.tile_pool(name="dram", bufs=1, space="DRAM") as dram,
            tc.tile_pool(name="mpp", bufs=1, space="PSUM") as mpp,
            tc.tile_pool(name="msb", bufs=2) as msb,
            tc.tile_pool(name="zp", bufs=2, space="PSUM") as zp,
            tc.tile_pool(name="lgp", bufs=1, space="PSUM") as lgp,
            tc.tile_pool(name="accp", bufs=1, space="PSUM") as accp,
            tc.tile_pool(name="denp", bufs=1, space="PSUM") as denp,
            tc.tile_pool(name="trp", bufs=2, space="PSUM") as trp,
            tc.tile_pool(name="esb", bufs=2) as esb,
            tc.tile_pool(name="dbgp", bufs=1) as dbgp,
        ):
            nc.gpsimd.load_library(mlp_lib)
            iota = cpool.tile([128, 128], BF16)
            nc.sync.dma_start(out=iota[:], in_=iota_in[:, :])
            ident = cpool.tile([128, 128], BF16)
            nc.sync.dma_start(out=ident[:], in_=id_in[:, :])
            wlr = cpool.tile([FH, L * 2 * FH], BF16)
            nc.sync.dma_start(out=wlr[:], in_=wlr_in[:, :])
            ab = cpool.tile([FH, L * H], BF16)
            nc.sync.dma_start(out=ab[:], in_=ab_in[:, :])
            blr = cpool.tile([1, L * 2 * FH], F32)
            nc.sync.dma_start(out=blr[:], in_=blr_in[:, :])
            bo = cpool.tile([128, L * FH], F32)
            nc.sync.dma_start(out=bo[:], in_=bo_in[:, :])
            wfb = cpool.tile([128, FH], F32)
            nc.sync.dma_start(out=wfb[:], in_=wf_in[:, :])
            bfb = cpool.tile([128, 1], F32)
            nc.sync.dma_start(out=bfb[:], in_=bf_in[:, :])
            ones1 = cpool.tile([1, 128], F32)
            nc.vector.memset(ones1[:], 1.0)

            hbuf = [bigp.tile([128, NPC], BF16, tag=f"h{i}", name=f"h{i}")
                    for i in range(2)]
            xr_pin = [bigp.tile([128, NPC], BF16, tag=f"xrp{i}",
                                name=f"xrp{i}") for i in range(2)]
            nc.gpsimd.dma_start(out=hbuf[0][:], in_=x_in[:, :])

            xl_shards = [dram.tile([NPC, FH], BF16, name=f"xl_shard{i}")
                         for i in range(L)]
            xl_fulls = [dram.tile([NTAB, FH], BF16,
                                  name=f"xl_full{i}") for i in range(L)]
            h3f = bigp.tile([128, NPC], F32, tag="h3f")

            def emit_m_blocks(layer, b0, nblk):
                """Node transform for blocks [b0, b0+nblk) of `layer`:
                reads hbuf[layer%2], writes xr_pin[layer%2] and
                xl_shards[layer] (via SBUF staging + DMA)."""
                h = hbuf[layer % 2]
                xrp = xr_pin[layer % 2]
                shard = xl_shards[layer]
                hT_ps = trp.tile([128, 8, 128], BF16, tag="trp")
                for j in range(nblk):
                    b = b0 + j
                    nc.tensor.transpose(
                        out=hT_ps[:, j, :], in_=h[:, b * 128:(b + 1) * 128],
                        identity=ident[:])
                hT = msb.tile([128, nblk, 128], BF16, tag="hT")
                nc.scalar.copy(hT[:], hT_ps[:, 0:nblk, :])
                ps = mpp.tile([128, nblk, 256], F32, tag="xlr")
                for j in range(nblk):
                    nc.tensor.matmul(
                        out=ps[:, j, :], lhsT=hT[:, j, :],
                        rhs=wlr[:, layer * 2 * FH:(layer + 1) * 2 * FH],
                        start=True, stop=False)
                    nc.tensor.matmul(
                        out=ps[:, j, :], lhsT=ones1[:],
                        rhs=blr[:, layer * 2 * FH:(layer + 1) * 2 * FH],
                        start=False, stop=True)
                xl_sb = msb.tile([128, nblk, 128], BF16, tag="xlsb")
                nc.scalar.copy(xl_sb[:], ps[:, :, 0:128])
                nc.scalar.copy(xrp[:, b0 * 128:(b0 + nblk) * 128],
                               ps[:, :, 128:256])
                for j in range(nblk):
                    nc.sync.dma_start(
                        out=shard[(b0 + j) * 128:(b0 + j + 1) * 128, :],
                        in_=xl_sb[:, j, :])
                if DEBUG and layer == 0 and b0 == 0:
                    nc.sync.dma_start(
                        out=dbg["xls"][:, :],
                        in_=xl_sb[:].rearrange("p j c -> p (j c)"))
                    nc.sync.dma_start(out=dbg["xrp"][:, :],
                                      in_=xrp[:, 0:nblk * 128])

            def emit_ag_chunk(layer, k):
                rows = BPC * 128
                nc.gpsimd.collective_compute(
                    "AllGather", OP.bypass,
                    replica_groups=[list(range(NCORES))],
                    ins=[xl_shards[layer][k * rows:(k + 1) * rows, :].opt()],
                    outs=[xl_fulls[layer]
                          [k * NCORES * rows:(k + 1) * NCORES * rows,
                           :].opt()],
                )

            # layer 0 node transforms + chunked AllGather
            for k in range(G):
                for j in range(BPC // 2):
                    emit_m_blocks(0, k * BPC + 2 * j, 2)
                emit_ag_chunk(0, k)

            for layer in range(L):
                last_layer = layer == L - 1
                xl_full = xl_fulls[layer]
                xrp = xr_pin[layer % 2]
                hn = hbuf[(layer + 1) % 2]

                for sc in range(NSC):
                    itlo = esb.tile([128, C * TLO * 128 // 16], I16,
                                    tag="itlo")
                    nc.sync.dma_start(out=itlo[:], in_=ilo[sc])
                    if THI:
                        ithi = esb.tile([128, C * THI * 128 // 16], I16,
                                        tag="ithi")
                        nc.sync.dma_start(out=ithi[:], in_=ihi[sc])
                    dl = esb.tile([128, CT], BF16, tag="dl")
                    nc.sync.dma_start(out=dl[:], in_=dstl[sc])

                    # xl gather (edge-major, HBM source)
                    xln = esb.tile([128, CT * 128], BF16, tag="xln")
                    xln3 = xln[:].rearrange("p (t c) -> p t c", c=128)
                    parts = [(0, C * TLO // 2, itlo, 0, 0),
                             (C * TLO // 2, C * TLO - C * TLO // 2, itlo,
                              C * TLO // 2, 1)]
                    if THI:
                        parts.append((C * TLO, C * THI, ithi, 0, 2))
                    for base, n, it, io, q in parts:
                        ne = n * 128
                        srcap = (xl_full[:, :] if base < C * TLO
                                 else xl_full[cfg.BBASE:, :])
                        nc.gpsimd.dma_gather(
                            out_ap=xln3[:, base:base + n, :],
                            in_ap=srcap,
                            idxs_ap=it[:, io * 8:(io + n) * 8],
                            num_idxs=ne, num_idxs_reg=ne,
                            elem_size=FH,
                            queue_num=q,
                            single_packet=False)

                    # one-hots for all tiles in one DVE op (1x mode;
                    # the dl comparand broadcast blocks 2x)
                    oh_all = esb.tile([128, CT * 128], BF16, tag="oh")
                    nc.vector.tensor_tensor(
                        out=oh_all[:].rearrange(
                            "p (t c) -> p t c", c=128),
                        in0=iota[:][:, None, :].to_broadcast(
                            [128, CT, 128]),
                        in1=dl[:, :, None].to_broadcast([128, CT, 128]),
                        op=OP.is_equal)

                    # transposed one-hots (PE) for the xr expand
                    ohT_sb = esb.tile([128, CT, 128], BF16, tag="ohT")
                    n8 = math.ceil(CT / 8)
                    for g in range(n8):
                        t0, t1 = g * 8, min(g * 8 + 8, CT)
                        ohT_ps = trp.tile([128, 8, 128], BF16, tag="trp")
                        for t in range(t0, t1):
                            nc.tensor.transpose(
                                out=ohT_ps[:, t - t0, :],
                                in_=oh_all[:, t * 128:(t + 1) * 128],
                                identity=ident[:])
                        nc.scalar.copy(
                            ohT_sb[:, t0:t1, :], ohT_ps[:, 0:t1 - t0, :])

                    def blk_of(t):
                        if t < C * TLO:
                            j = t // TLO
                            first = (t % TLO) == 0
                            last = THI == 0 and (t % TLO) == TLO - 1
                        else:
                            j = (t - C * TLO) // THI
                            first = False
                            last = ((t - C * TLO) % THI) == THI - 1
                        return j, first, last

                    # z_pre = xl^T + xr via identity-weight matmuls; leaky
                    zT_sb = esb.tile([128, CT, 128], BF16, tag="zT")
                    for g in range(NZG):
                        t0, t1 = g * TG, min(g * TG + TG, CT)
                        zpre = zp.tile([128, TG, 128], F32, tag="zpre")
                        for t in range(t0, t1):
                            j, _, _ = blk_of(t)
                            bcol = (sc * C + j) * 128
                            nc.tensor.matmul(
                                out=zpre[:, t - t0, :],
                                lhsT=xrp[:, bcol:bcol + 128],
                                rhs=ohT_sb[:, t, :],
                                start=True, stop=False)
                            nc.tensor.matmul(
                                out=zpre[:, t - t0, :],
                                lhsT=xln3[:, t, :], rhs=ident[:],
                                start=False, stop=True)
                        nc.scalar.activation(
                            zT_sb[:, t0:t1, :],
                            zpre[:, 0:t1 - t0, :],
                            AF.Prelu, alpha=0.2)

                    # logits; exp
                    lgs = esb.tile([128, CT * H], BF16, tag="lgs")
                    nsub = math.ceil(CT / 16)
                    for si in range(nsub):
                        t0, t1 = si * 16, min(si * 16 + 16, CT)
                        nt = t1 - t0
                        lg = lgp.tile([128, 64], F32, tag="lg")
                        for t in range(t0, t1):
                            nc.tensor.matmul(
                                out=lg[:, (t - t0) * 4:(t - t0) * 4 + 4],
                                lhsT=zT_sb[:, t, :],
                                rhs=ab[:, layer * H:(layer + 1) * H],
                                start=True, stop=True)
                        nc.scalar.activation(
                            lgs[:, t0 * H:t1 * H], lg[:, 0:nt * 4], AF.Exp)

                    # mw = xln * explg (gpsimd: frees DVE; 1x either way)
                    mw = esb.tile([128, CT * 128], BF16, tag="mw")
                    nc.gpsimd.tensor_mul(
                        mw[:].rearrange("p (th c) -> p th c", c=HID),
                        xln[:].rearrange("p (th c) -> p th c", c=HID),
                        lgs[:][:, :, None].to_broadcast(
                            [128, CT * H, HID]))

                    # segment sums via one-hot matmuls (acc cols
                    # 0:128 = weighted messages, 128:132 = softmax denoms)
                    acc = accp.tile([128, C, 128], F32, tag="acc")
                    den = denp.tile([128, C, 4], F32, tag="den")
                    for j in range(C):
                        tiles = (list(range(j * TLO, (j + 1) * TLO))
                                 + list(range(C * TLO + j * THI,
                                              C * TLO + (j + 1) * THI)))
                        for i, t in enumerate(tiles):
                            first = i == 0
                            last = i == len(tiles) - 1
                            nc.tensor.matmul(
                                out=acc[:, j, :],
                                lhsT=oh_all[:, t * 128:(t + 1) * 128],
                                rhs=mw[:, t * 128:(t + 1) * 128],
                                start=first, stop=last)
                            nc.tensor.matmul(
                                out=den[:, j, :],
                                lhsT=oh_all[:, t * 128:(t + 1) * 128],
                                rhs=lgs[:, t * H:(t + 1) * H],
                                start=first, stop=last)

                    if DEBUG and layer == 0 and sc == 0:
                        dpool = dbgp
                        for nm, srcap in (
                            ("xln", xln[:]),
                            ("zT", zT_sb[:].rearrange(
                                "p t c -> p (t c)")),
                            ("mw", mw[:]),
                            ("oh", oh_all[:]),
                        ):
                            nc.sync.dma_start(out=dbg[nm][:, :], in_=srcap)
                        acs = dbgp.tile([128, C * 128], F32, tag="dbgacc")
                        nc.vector.tensor_copy(
                            acs[:], acc[:].rearrange("p j c -> p (j c)"))
                        nc.sync.dma_start(out=dbg["acc"][:, :], in_=acs[:])
                        dns = dbgp.tile([128, C * 4], F32, tag="dbgden")
                        nc.vector.tensor_copy(
                            dns[:], den[:].rearrange("p j c -> p (j c)"))
                        nc.sync.dma_start(out=dbg["den"][:, :], in_=dns[:])

                    # epilogue: normalize, +bo, ELU -> hn
                    asb = esb.tile([128, C, 128], F32, tag="asb")
                    nc.vector.tensor_copy(asb[:], acc[:])
                    rec = esb.tile([128, C * 4], F32, tag="rec")
                    nc.vector.tensor_scalar_max(
                        rec[:].rearrange("p (j h) -> p j h", j=C),
                        den[:], 1e-16)
                    nc.vector.reciprocal(rec[:], rec[:])
                    u = esb.tile([128, C * 128], F32, tag="u")
                    nc.vector.tensor_mul(
                        u[:].rearrange("p (j h c) -> p j h c", j=C, h=H),
                        asb[:].rearrange(
                            "p j (h c) -> p j h c", h=H),
                        rec[:].rearrange("p (j h) -> p j h", j=C)
                        [:, :, :, None].to_broadcast([128, C, H, HID]))
                    nc.vector.tensor_add(
                        u[:].rearrange("p (j f) -> p j f", j=C),
                        u[:].rearrange("p (j f) -> p j f", j=C),
                        bo[:, layer * FH:(layer + 1) * FH]
                        [:, None, :].to_broadcast([128, C, FH]))
                    # elu(u) = (exp(min(u,0)) - 1) + relu(u)
                    r = esb.tile([128, C * 128], F32, tag="r")
                    nc.vector.tensor_scalar_max(r[:], u[:], 0.0)
                    tmin = esb.tile([128, C * 128], F32, tag="tmin")
                    nc.vector.tensor_scalar_min(tmin[:], u[:], 0.0)
                    s_ = esb.tile([128, C * 128], F32, tag="s")
                    nc.scalar.activation(s_[:], tmin[:], AF.Exp)
                    hdst = h3f if last_layer else hn
                    nc.vector.scalar_tensor_tensor(
                        out=hdst[:, sc * C * 128:(sc + 1) * C * 128],
                        in0=s_[:], scalar=-1.0, in1=r[:],
                        op0=OP.add, op1=OP.add)
                    if DEBUG and layer == 0 and sc == 0:
                        nc.sync.dma_start(
                            out=dbg["hn"][:, :],
                            in_=hn[:, sc * C * 128:(sc + 1) * C * 128])

                    # interleave next layer's node transforms + AG chunks
                    if not last_layer:
                        emit_m_blocks(layer + 1, sc * C, C)
                        if (sc + 1) % SCPC == 0:
                            emit_ag_chunk(layer + 1, (sc + 1) // SCPC - 1)

            if DEBUG:
                nc.sync.dma_start(out=dbg["xlf"][:, :],
                                  in_=xl_fulls[0][:, :])
                nc.sync.dma_start(out=dbg["h0"][:, :], in_=hbuf[1][:])
                nc.sync.dma_start(out=dbg["h1"][:, :], in_=hbuf[0][:])
                nc.sync.dma_start(out=dbg["h2"][:, :], in_=h3f[:])
            # ------------- final linear (f32, chunked to bound scratch)
            FB = 5
            with (
                tc.tile_pool(name="fin", bufs=2) as fin,
                tc.tile_pool(name="fino", bufs=1) as fino,
            ):
                of = fino.tile([128, NB], F32)
                for fb in range(0, NB, FB):
                    fm = fin.tile([128, FB, 128], F32, tag="fm")
                    nc.vector.tensor_mul(
                        fm[:],
                        h3f[:, fb * 128:(fb + FB) * 128].rearrange(
                            "p (b f) -> p b f", b=FB),
                        wfb[:][:, None, :].to_broadcast([128, FB, FH]))
                    nc.vector.tensor_reduce(
                        out=of[:, fb:fb + FB], in_=fm[:],
                        axis=mybir.AxisListType.X, op=OP.add)
                nc.vector.tensor_scalar_add(of[:], of[:], bfb[:, 0:1])
                nc.sync.dma_start(out=out_t[:, :], in_=of[:])

    nc.compile()
    return nc


# ---------------------------------------------------------------- inputs


def _to_bf(x):
    return np.asarray(x, np.float32).astype(BF)


def make_inputs(cfg, pre, inputs):
    NB, NPC = cfg.NB, cfg.NPC
    x = np.asarray(inputs["x"], np.float32)
    xp_all = np.zeros((cfg.NTAB, IN), np.float32)
    xp_all[pre["perm_pos"]] = x
    W_l = np.stack([inputs["W_l0"], *[inputs["W_l"][i] for i in range(L - 1)]])
    W_r = np.stack([inputs["W_r0"], *[inputs["W_r"][i] for i in range(L - 1)]])
    att = np.stack([inputs["att0"], *[inputs["att"][i] for i in range(L - 1)]])
    b_l = np.stack([inputs["b_l0"], *[inputs["b_l"][i] for i in range(L - 1)]])
    b_r = np.stack([inputs["b_r0"], *[inputs["b_r"][i] for i in range(L - 1)]])
    bo = np.stack([inputs["bo0"], *[inputs["bo"][i] for i in range(L - 1)]])
    ablk = np.stack([make_ablk(att[l]) for l in range(L)])
    blr = np.stack([b_l, b_r], axis=1).astype(np.float32)
    bo_b = np.repeat(np.asarray(bo, np.float32)[:, None, :], 128, axis=1)
    wf = np.asarray(inputs["W_f"], np.float32)
    wfb = np.repeat(wf[:, 0][None, :], 128, axis=0)
    bfb = np.full((128, 1), float(np.asarray(inputs["b_f"]).ravel()[0]),
                  np.float32)
    iota = np.repeat(np.arange(128, dtype=np.float32)[None, :], 128, axis=0)
    ident = np.eye(128, dtype=np.float32)

    wlr_p = np.concatenate(
        [np.concatenate([W_l[l], W_r[l]], axis=1) for l in range(L)], axis=1)
    ab_p = np.concatenate([ablk[l] for l in range(L)], axis=1)   # [FH, L*H]
    blr_p = blr.reshape(1, -1).astype(np.float32)                # [1, L*2*FH]
    bo_p = np.concatenate([bo_b[l] for l in range(L)], axis=1)   # [128, L*FH]
    shared = dict(
        wlr=_to_bf(wlr_p), ab=_to_bf(ab_p), blr=blr_p,
        bo=bo_p.astype(np.float32), wfb=wfb.astype(np.float32), bfb=bfb,
        iota=_to_bf(iota), ident=_to_bf(ident),
    )
    # per-core local table: node (core, b, r) -> xp row r, col b*IN..
    rows = np.arange(cfg.NTAB)
    # invert chunk-major layout: for each table row, which (core,b,r)?
    ch = rows // (NCORES * cfg.BPC * 128)
    rem = rows % (NCORES * cfg.BPC * 128)
    core_of = rem // (cfg.BPC * 128)
    rem2 = rem % (cfg.BPC * 128)
    b_of = ch * cfg.BPC + rem2 // 128
    r_of = rem2 % 128
    in_maps = []
    for c in range(NCORES):
        sel = core_of == c
        xp_t = np.zeros((128, NB, IN), np.float32)
        xp_t[r_of[sel], b_of[sel]] = xp_all[rows[sel]]
        in_maps.append(dict(
            xp=_to_bf(xp_t.reshape(128, NB * IN)),
            ilo=pre["idx_lo"][c], ihi=pre["idx_hi"][c],
            ixr=pre["idx_xr"][c],
            dstl=pre["dstloc"][c],
            **shared,
        ))
    return in_maps


def assemble_output(cfg, pre, out_maps):
    full = np.zeros((cfg.NTAB,), np.float32)
    for c in range(NCORES):
        o = np.asarray(out_maps[c]["out"], np.float32)  # [128 r, NB b]
        b = np.arange(cfg.NB)
        r = np.arange(128)
        rr, bb = np.meshgrid(r, b, indexing="ij")
        trow = table_row(cfg, c, bb, rr)
        full[trow.reshape(-1)] = o.reshape(-1)
    return full[pre["perm_pos"]][:, None].astype(np.float32)


# ---------------------------------------------------------------- runner


class CompiledSPMD:
    """Compile the bass module once; run it many times on n_cores devices."""

    def __init__(self, nc, n_cores):
        import jax
        from jax.sharding import Mesh, PartitionSpec
        from jax.experimental.shard_map import shard_map
        from concourse import bass2jax
        from concourse.bass2jax import _bass_exec_p, install_neuronx_cc_hook
        self._jax = jax
        install_neuronx_cc_hook()
        self.nc = nc
        self.n_cores = n_cores
        partition_name = (nc.partition_id_tensor.name
                          if nc.partition_id_tensor else None)
        in_names, out_names, out_avals, zero_outs = [], [], [], []
        for alloc in nc.m.functions[0].allocations:
            if not isinstance(alloc, mybir.MemoryLocationSet):
                continue
            name = alloc.memorylocations[0].name
            if alloc.kind == "ExternalInput":
                if name != partition_name and name != (
                        nc.dbg_addr.name if nc.dbg_addr else None):
                    in_names.append(name)
            elif alloc.kind == "ExternalOutput":
                out_names.append(name)
                shape = tuple(alloc.tensor_shape)
                dtype = mybir.dt.np(alloc.dtype)
                out_avals.append(jax.core.ShapedArray(shape, dtype))
                zero_outs.append(np.zeros(shape, dtype))
        self.in_names, self.out_names = in_names, out_names
        self.out_avals, self.zero_outs = out_avals, zero_outs
        n_params, n_outs = len(in_names), len(out_names)
        all_in = list(in_names) + list(out_names)
        if nc.dbg_addr is not None:
            all_in.append(nc.dbg_addr.name)
        if partition_name is not None:
            all_in.append(partition_name)
        dbg_name = nc.dbg_addr.name if nc.dbg_addr is not None else None

        def _body(*args):
            operands = list(args)
            if dbg_name is not None:
                operands.append(jax.numpy.zeros((1, 2), jax.numpy.uint32))
            if partition_name is not None:
                operands.append(bass2jax.partition_id_tensor())
            outs = _bass_exec_p.bind(
                *operands, out_avals=tuple(out_avals),
                in_names=tuple(all_in), out_names=tuple(out_names),
                lowering_input_output_aliases=(),
                sim_require_finite=True, sim_require_nnan=True, nc=nc)
            return tuple(outs)

        devices = jax.devices()[:n_cores]
        assert len(devices) == n_cores
        self._mesh = Mesh(np.asarray(devices), ("core",))
        in_specs = (PartitionSpec("core"),) * (n_params + n_outs)
        out_specs = (PartitionSpec("core"),) * n_outs
        self._P = PartitionSpec
        self._fn = jax.jit(
            shard_map(_body, mesh=self._mesh, in_specs=in_specs,
                      out_specs=out_specs, check_rep=False),
            keep_unused=True)

    def prepare_inputs(self, in_maps):
        jax = self._jax
        assert len(in_maps) == self.n_cores
        concat_in = [
            np.concatenate([np.asarray(in_maps[c][n])
                            for c in range(self.n_cores)], axis=0)
            for n in self.in_names]
        concat_zeros = [
            np.zeros((self.n_cores * z.shape[0], *z.shape[1:]), z.dtype)
            for z in self.zero_outs]
        sh = jax.sharding.NamedSharding(self._mesh, self._P("core"))
        args = [jax.device_put(a, sh) for a in concat_in + concat_zeros]
        jax.block_until_ready(args)
        return args

    def run_to_maps(self, args):
        jax = self._jax
        outs = jax.block_until_ready(self._fn(*args))
        return [
            {name: np.asarray(outs[i]).reshape(
                self.n_cores, *self.out_avals[i].shape)[c]
             for i, name in enumerate(self.out_names)}
            for c in range(self.n_cores)]


_COMPILED = {}


def kernel(**inputs):
    cfg = FULL
    pre = preprocess(cfg, np.asarray(inputs["edge_index"]))
    key = (cfg.N, pre["TLO"], pre["THI"])
    if key not in _COMPILED:
        nc = build_program(cfg, pre["TLO"], pre["THI"])
        _COMPILED[key] = CompiledSPMD(nc, NCORES)
    comp = _COMPILED[key]
    in_maps = make_inputs(cfg, pre, inputs)
    args = comp.prepare_inputs(in_maps)
    out_maps = comp.run_to_maps(args)
    return assemble_output(cfg, pre, out_maps)


# revision 27
# speedup vs baseline: 2.0791x; 2.0791x over previous
"""GATv2 3-layer GNN on 8 Trainium2 NeuronCores.

Strategy (dst-sharded edge processing, single gather per edge):
- Nodes are bin-packed by in-degree into 8*NB bins of <=128 dst nodes each,
  balancing edges per bin. Bin -> (core, block). The xl feature table is
  stored in permuted order (core-major, block-major, row).
- Per layer, per core: xl/xr for own nodes via PE (transpose + matmul);
  xl shards AllGather'ed into a full table (bf16); xr kept in SBUF.
- Edges (grouped by dst block, padded to uniform tile counts) are processed
  in super-chunks of C blocks: ONE dma_gather of xl rows per edge; xr per
  edge comes from a one-hot-transpose matmul against the dst block's xr
  rows; xl is moved to channel-major via identity-RHS matmuls accumulating
  into the same PSUM (z_pre = xr_expand + xl^T). Leaky-relu on ScalarE,
  logits via PE (ldweights = z^T trick), exp+broadcast on ScalarE,
  messages w*xl on VectorE, segment-sum via one-hot matmuls into PSUM.
- Segment softmax skips max-subtraction (logits are O(1) by construction;
  exact same math, exp is safe in fp32).
- int16 gather indices: edges are split per block into a "lo" group
  (table rows < 32768, base 0) and "hi" group (rows >= BBASE, base BBASE),
  with flexible rows in [BBASE, 32768) used to balance the two groups.
"""
import math
import numpy as np
import ml_dtypes

import concourse.bacc as bacc
import concourse.bass as bass
import concourse.mybir as mybir
import concourse.tile as tile
from concourse.library_config import mlp as mlp_lib


# --- patch Tile's DMASW lane assignment to be SWDGE-queue-aware: a DMA sem
# lane must only ever be updated from one SWDGE queue; Tile round-robins
# lanes obliviously. Pin lanes {2q, 2q+1} to queue q (NQ<=4).
from concourse import tile_sem_assignment as _tsa


def _queue_aware_assign_tick(self, inst, *, _orig=_tsa.TileClockTick._assign_tick):
    if (isinstance(inst, _tsa.DMAInst)
            and inst.engine == mybir.EngineType.Pool):
        q = int(getattr(inst, "queue_num", 0) or 0)
        if not hasattr(self, "_qtog"):
            self._qtog = {}
        t = self._qtog.get(q, 0)
        lanes = max(1, self.swdge_sem_count // 4)
        self.next_sw_dma_idx = (q * lanes + t) % self.swdge_sem_count
        self._qtog[q] = (t + 1) % lanes
    return _orig(self, inst)


_tsa.TileClockTick._assign_tick = _queue_aware_assign_tick

F32 = mybir.dt.float32
BF16 = mybir.dt.bfloat16
I16 = mybir.dt.int16
BF = ml_dtypes.bfloat16
AF = mybir.ActivationFunctionType
OP = mybir.AluOpType

NCORES = 8
IN, HID, H, OUT, NLAYERS = 128, 32, 4, 1, 3
FH = HID * H  # 128
L = NLAYERS


class Cfg:
    def __init__(self, N, E_raw, NB, C=2, SPLIT=32768, NQ=4):
        self.N = N
        self.NB = NB            # blocks (bins) per core
        self.C = C              # blocks per super-chunk
        assert NB % C == 0
        self.NSC = NB // C
        self.NPC = NB * 128     # table rows per core
        self.NTAB = NCORES * self.NPC
        self.SPLIT = min(SPLIT, self.NTAB)
        self.BBASE = max(0, self.NTAB - self.SPLIT)  # hi-group table base
        self.NQ = NQ
        assert self.NTAB - self.BBASE <= self.SPLIT


FULL = Cfg(N=50000, E_raw=800000, NB=50)


# ---------------------------------------------------------------- host side


def _wrap_idx16(idx, pad_to):
    """[n] ints -> [128, pad_to//16] int16 dma_gather index layout
    (i at partition i%16, col i//16; replicated into all 8 groups)."""
    a = np.zeros(pad_to, np.int64)
    a[: len(idx)] = idx
    w = a.reshape(pad_to // 16, 16).T.astype(np.int16)
    return np.tile(w, (8, 1))


def preprocess(cfg, edge_index):
    N, NB, C = cfg.N, cfg.NB, cfg.C
    nbins = NCORES * NB
    src = np.concatenate([np.asarray(edge_index[0], np.int64),
                          np.arange(N, dtype=np.int64)])
    dst = np.concatenate([np.asarray(edge_index[1], np.int64),
                          np.arange(N, dtype=np.int64)])
    deg = np.bincount(dst, minlength=N)

    # snake-deal nodes (sorted by in-degree desc) into bins
    order = np.argsort(-deg, kind="stable")
    nrounds = math.ceil(N / nbins)
    binof = np.empty(N, np.int64)
    rowof = np.empty(N, np.int64)
    for r in range(nrounds):
        chunk = order[r * nbins:(r + 1) * nbins]
        cols = np.arange(len(chunk))
        if r % 2:
            cols = nbins - 1 - cols
        binof[chunk] = cols
        rowof[chunk] = r
    assert rowof.max() <= 127
    perm_pos = binof * 128 + rowof          # node -> table row

    psrc = perm_pos[src]
    ebin = binof[dst]
    erow = rowof[dst]

    eorder = np.argsort(ebin, kind="stable")
    psrc = psrc[eorder]
    erow = erow[eorder]
    counts = np.bincount(ebin[eorder], minlength=nbins)
    starts = np.concatenate([[0], np.cumsum(counts)])

    # lo/hi split with flexible band
    grp = np.zeros(len(psrc), np.int8)
    nlo = np.zeros(nbins, np.int64)
    for b in range(nbins):
        s, e = starts[b], starts[b + 1]
        p = psrc[s:e]
        lofix = p < cfg.BBASE
        hifix = p >= cfg.SPLIT
        flex = ~lofix & ~hifix
        a, bb, f = int(lofix.sum()), int(hifix.sum()), int(flex.sum())
        x = int(np.clip((bb + f - a + 1) // 2, 0, f))
        g = np.zeros(e - s, np.int8)
        g[hifix] = 1
        fi = np.nonzero(flex)[0]
        g[fi[x:]] = 1
        grp[s:e] = g
        nlo[b] = a + x
    nhi = counts - nlo
    if cfg.NTAB <= cfg.SPLIT:
        grp[:] = 0
        nlo = counts.copy()
        nhi[:] = 0
    TLO = max(1, int(np.ceil(nlo.max() / 128)))
    THI = int(np.ceil(nhi.max() / 128))
    TT = TLO + THI

    idx_lo = np.zeros((NCORES, cfg.NSC, 128, C * TLO * 128 // 16), np.int16)
    idx_hi = np.zeros((NCORES, cfg.NSC, 128, max(1, C * THI * 128 // 16)),
                      np.int16)
    dstloc = np.full((NCORES, cfg.NSC, C * TT, 128), 255.0, np.float32)

    for core in range(NCORES):
        for sc in range(cfg.NSC):
            blocks = [core * NB + sc * C + j for j in range(C)]
            lo_list, hi_list = [], []
            for j, b in enumerate(blocks):
                s, e = starts[b], starts[b + 1]
                g = grp[s:e]
                p = psrc[s:e].copy()
                r = erow[s:e]
                for gi, (tbase, tcnt, lst) in enumerate(
                    ((0, TLO, lo_list), (C * TLO, THI, hi_list))
                ):
                    sel = g == gi
                    pp = p[sel]
                    rr = r[sel]
                    if gi == 1:
                        pp = pp - cfg.BBASE
                    assert len(pp) <= tcnt * 128
                    pad = tcnt * 128 - len(pp)
                    ppad = np.concatenate([pp, np.zeros(pad, np.int64)])
                    dpad = np.concatenate(
                        [rr.astype(np.float32),
                         np.full(pad, 255.0, np.float32)])
                    lst.append(ppad)
                    for t in range(tcnt):
                        gt = tbase + j * tcnt + t
                        dstloc[core, sc, gt] = dpad[t * 128:(t + 1) * 128]
            idx_lo[core, sc] = _wrap_idx16(np.concatenate(lo_list),
                                           C * TLO * 128)
            if THI:
                idx_hi[core, sc] = _wrap_idx16(np.concatenate(hi_list),
                                               C * THI * 128)

    # dstloc -> [128 edge-row, C*TT] per (core, sc)
    dstloc = np.ascontiguousarray(dstloc.transpose(0, 1, 3, 2))
    return dict(perm_pos=perm_pos, TLO=TLO, THI=THI,
                idx_lo=idx_lo, idx_hi=idx_hi, dstloc=dstloc.astype(BF))


def make_ablk(att_l):  # [H, HID] -> [FH, H]
    a = np.zeros((FH, H), np.float32)
    for h in range(H):
        a[h * HID:(h + 1) * HID, h] = att_l[h]
    return a


# ---------------------------------------------------------------- program


def build_program(cfg, TLO, THI, reps=1, ablate="none"):
    nc = bacc.Bacc("TRN2", target_bir_lowering=False, debug=False,
                   num_devices=NCORES, num_swdge_queues=cfg.NQ)
    NB, C, NSC, NPC, NTAB = cfg.NB, cfg.C, cfg.NSC, cfg.NPC, cfg.NTAB
    TT = TLO + THI
    CT = C * TT
    CE = CT * 128

    x_in = nc.dram_tensor("xp", [128, NPC], BF16, kind="ExternalInput")
    ilo = nc.dram_tensor("ilo", [NSC, 128, C * TLO * 128 // 16], I16,
                         kind="ExternalInput")
    ihi = nc.dram_tensor("ihi", [NSC, 128, max(1, C * THI * 128 // 16)], I16,
                         kind="ExternalInput")
    dstl = nc.dram_tensor("dstl", [NSC, 128, CT], BF16, kind="ExternalInput")
    wlr_in = nc.dram_tensor("wlr", [FH, L * 2 * FH], BF16,
                            kind="ExternalInput")
    ab_in = nc.dram_tensor("ab", [FH, L * H], BF16, kind="ExternalInput")
    blr_in = nc.dram_tensor("blr", [1, L * 2 * FH], F32, kind="ExternalInput")
    bo_in = nc.dram_tensor("bo", [128, L * FH], F32, kind="ExternalInput")
    wf_in = nc.dram_tensor("wfb", [128, FH], F32, kind="ExternalInput")
    bf_in = nc.dram_tensor("bfb", [128, 1], F32, kind="ExternalInput")
    iota_in = nc.dram_tensor("iota", [128, 128], BF16, kind="ExternalInput")
    id_in = nc.dram_tensor("ident", [128, 128], BF16, kind="ExternalInput")
    out_t = nc.dram_tensor("out", [128, NB], F32, kind="ExternalOutput")

    with tile.TileContext(nc) as tc:
        with (
            tc.tile_pool(name="const", bufs=1) as cpool,
            tc.tile_pool(name="big", bufs=1) as bigp,
            tc.tile_pool(name="dram", bufs=1, space="DRAM") as dram,
        ):
            nc.gpsimd.load_library(mlp_lib)
            iota = cpool.tile([128, 128], BF16)
            nc.sync.dma_start(out=iota[:], in_=iota_in[:, :])
            ident = cpool.tile([128, 128], BF16)
            nc.sync.dma_start(out=ident[:], in_=id_in[:, :])
            wlr = cpool.tile([FH, L * 2 * FH], BF16)
            nc.sync.dma_start(out=wlr[:], in_=wlr_in[:, :])
            ab = cpool.tile([FH, L * H], BF16)
            nc.sync.dma_start(out=ab[:], in_=ab_in[:, :])
            blr = cpool.tile([1, L * 2 * FH], F32)
            nc.sync.dma_start(out=blr[:], in_=blr_in[:, :])
            bo = cpool.tile([128, L * FH], F32)
            nc.sync.dma_start(out=bo[:], in_=bo_in[:, :])
            wfb = cpool.tile([128, FH], F32)
            nc.sync.dma_start(out=wfb[:], in_=wf_in[:, :])
            bfb = cpool.tile([128, 1], F32)
            nc.sync.dma_start(out=bfb[:], in_=bf_in[:, :])
            ones1 = cpool.tile([1, 128], F32)
            nc.vector.memset(ones1[:], 1.0)

            hbuf = [bigp.tile([128, NPC], BF16, tag=f"h{i}", name=f"h{i}")
                    for i in range(2)]
            xr_pin = bigp.tile([128, NPC], BF16, tag="xrp")
            nc.gpsimd.dma_start(out=hbuf[0][:], in_=x_in[:, :])

            xl_shards = [dram.tile([NPC, FH], BF16, name=f"xl_shard{i}")
                         for i in range(L * reps)]
            xl_fulls = [dram.tile([NTAB, FH], BF16, addr_space="Shared",
                                  name=f"xl_full{i}") for i in range(L * reps)]
            h3f = bigp.tile([128, NPC], F32, tag="h3f")

            for rep in range(reps):
              if rep > 0:
                nc.gpsimd.dma_start(out=hbuf[0][:], in_=x_in[:, :])
              for layer in range(L):
                h = hbuf[layer % 2]
                hn = hbuf[(layer + 1) % 2]
                last_layer = layer == L - 1
                xl_shard = xl_shards[rep * L + layer]
                xl_full = xl_fulls[rep * L + layer]
                # ------------- phase M (batched: 4 blocks per group)
                with (
                    tc.tile_pool(name=f"mp{layer}", bufs=2, space="PSUM") as mp,
                    tc.tile_pool(name=f"mx{layer}", bufs=2, space="PSUM") as mx,
                    tc.tile_pool(name=f"ms{layer}", bufs=3) as msb,
                ):
                    for b0 in range(0, NB, 4):
                        nbk = min(4, NB - b0)
                        hT_ps = mp.tile([128, 4, 128], BF16, tag="hT")
                        for j in range(nbk):
                            b = b0 + j
                            nc.tensor.transpose(
                                out=hT_ps[:, j, :],
                                in_=h[:, b * 128:(b + 1) * 128],
                                identity=ident[:])
                        hT = msb.tile([128, 4, 128], BF16, tag="hT")
                        nc.scalar.copy(hT[:, 0:nbk, :], hT_ps[:, 0:nbk, :])
                        ps = [mx.tile([128, 2, 256], F32, tag=f"x{i}",
                                      name=f"x{i}") for i in range(2)]
                        for j in range(nbk):
                            nc.tensor.matmul(
                                out=ps[j // 2][:, j % 2, :],
                                lhsT=hT[:, j, :],
                                rhs=wlr[:, layer * 2 * FH:
                                        (layer + 1) * 2 * FH],
                                start=True, stop=False)
                            nc.tensor.matmul(
                                out=ps[j // 2][:, j % 2, :], lhsT=ones1[:],
                                rhs=blr[:, layer * 2 * FH:
                                        (layer + 1) * 2 * FH],
                                start=False, stop=True)
                        xl_sb = msb.tile([128, 4, 128], BF16, tag="xlsb")
                        for i in range((nbk + 1) // 2):
                            nj = min(2, nbk - 2 * i)
                            nc.scalar.copy(xl_sb[:, 2 * i:2 * i + nj, :],
                                           ps[i][:, 0:nj, 0:128])
                            nc.vector.tensor_copy(
                                xr_pin[:, (b0 + 2 * i) * 128:
                                       (b0 + 2 * i + nj) * 128]
                                .rearrange("p (j c) -> p j c", j=nj),
                                ps[i][:, 0:nj, 128:256])
                        for j in range(nbk):
                            nc.sync.dma_start(
                                out=xl_shard[(b0 + j) * 128:
                                             (b0 + j + 1) * 128, :],
                                in_=xl_sb[:, j, :])

                nc.gpsimd.collective_compute(
                    "AllGather", OP.bypass,
                    replica_groups=[list(range(NCORES))],
                    ins=[xl_shard.opt()], outs=[xl_full.opt()],
                )

                # ------------- phase E
                with (
                    tc.tile_pool(name=f"ea{layer}", bufs=1, space="PSUM") as accp,
                    tc.tile_pool(name=f"ez{layer}", bufs=2, space="PSUM") as zp,
                    tc.tile_pool(name=f"eo{layer}", bufs=2, space="PSUM") as otp,
                    tc.tile_pool(name=f"el{layer}", bufs=2, space="PSUM") as lgp,
                    tc.tile_pool(name=f"es{layer}", bufs=2) as esb,
                    tc.tile_pool(name=f"eoh{layer}", bufs=2) as ohp,
                ):
                    for sc in range(NSC):
                        itlo = esb.tile([128, C * TLO * 128 // 16], I16,
                                        tag="itlo")
                        nc.sync.dma_start(out=itlo[:], in_=ilo[sc])
                        if THI:
                            ithi = esb.tile([128, C * THI * 128 // 16], I16,
                                            tag="ithi")
                            nc.sync.dma_start(out=ithi[:], in_=ihi[sc])
                        dl = esb.tile([128, CT], BF16, tag="dl")
                        nc.sync.dma_start(out=dl[:], in_=dstl[sc])

                        xln = esb.tile([128, CT, 128], BF16, tag="xln")
                        nlo_e = C * TLO * 128
                        if ablate == "nogather":
                            nc.vector.memset(xln[:, 0:1, :], 0.5)
                        else:
                            qb = 0
                            for base, ntile, it in (
                                (0, C * TLO, itlo),
                                (C * TLO, C * THI, ithi if THI else None),
                            ):
                                if not ntile:
                                    continue
                                srcap = (xl_full[:, :] if base == 0
                                         else xl_full[cfg.BBASE:, :])
                                half = ntile // 2
                                parts = ([(0, half), (half, ntile - half)]
                                         if half else [(0, ntile)])
                                for (o, n) in parts:
                                    ne = n * 128
                                    # idx slice: 8 columns per tile
                                    nc.gpsimd.dma_gather(
                                        out_ap=xln[:, base + o:base + o + n, :],
                                        in_ap=srcap,
                                        idxs_ap=it[:, o * 8:(o + n) * 8],
                                        num_idxs=ne, num_idxs_reg=ne,
                                        elem_size=FH,
                                        queue_num=(4 * sc + qb) % cfg.NQ,
                                        single_packet=False)
                                    qb += 1

                        if ablate == "nocompute":
                            nc.vector.memset(
                                hn[:, sc * C * 128:(sc + 1) * C * 128], 0.01)
                            if last_layer:
                                nc.vector.memset(
                                    h3f[:, sc * C * 128:(sc + 1) * C * 128],
                                    0.01)
                            continue

                        def blk_of(t):
                            if t < C * TLO:
                                j = t // TLO
                                first = (t % TLO) == 0
                                last = THI == 0 and (t % TLO) == TLO - 1
                            else:
                                j = (t - C * TLO) // THI
                                first = False
                                last = ((t - C * TLO) % THI) == THI - 1
                            return j, first, last

                        # one-hots (4 batched DVE builds) + transposes
                        OHB = 9
                        assert CT % OHB == 0
                        oh_all = ohp.tile([128, CT, 128], BF16, tag="oh")
                        for g in range(CT // OHB):
                            nc.vector.tensor_tensor(
                                out=oh_all[:, g * OHB:(g + 1) * OHB, :],
                                in0=iota[:][:, None, :].to_broadcast(
                                    [128, OHB, 128]),
                                in1=dl[:, g * OHB:(g + 1) * OHB, None]
                                .to_broadcast([128, OHB, 128]),
                                op=OP.is_equal)
                        ohs = [oh_all[:, t, :] for t in range(CT)]
                        n8 = math.ceil(CT / 8)
                        ohT_sb = esb.tile([128, CT, 128], BF16, tag="ohT")
                        for g in range(n8):
                            t0, t1 = g * 8, min(g * 8 + 8, CT)
                            ohT_ps = otp.tile([128, 8 * 128], BF16, tag="ohT")
                            for t in range(t0, t1):
                                nc.tensor.transpose(
                                    out=ohT_ps[:, (t - t0) * 128:
                                               (t - t0 + 1) * 128],
                                    in_=ohs[t], identity=ident[:])
                            nc.scalar.copy(
                                ohT_sb[:, t0:t1, :],
                                ohT_ps[:, :(t1 - t0) * 128])

                        # z_pre = xr_expand + xl^T ; leaky; z^T in SBUF
                        zT_sb = esb.tile([128, CT, 128], BF16, tag="zT")
                        n4 = math.ceil(CT / 4)
                        for g in range(n4):
                            t0, t1 = g * 4, min(g * 4 + 4, CT)
                            zpre = zp.tile([128, 4 * 128], F32, tag="zpre")
                            for t in range(t0, t1):
                                j, _, _ = blk_of(t)
                                bcol = (sc * C + j) * 128
                                o = (t - t0) * 128
                                nc.tensor.matmul(
                                    out=zpre[:, o:o + 128],
                                    lhsT=xr_pin[:, bcol:bcol + 128],
                                    rhs=ohT_sb[:, t, :],
                                    start=True, stop=False)
                                nc.tensor.matmul(
                                    out=zpre[:, o:o + 128],
                                    lhsT=xln[:, t, :], rhs=ident[:],
                                    start=False, stop=True)
                            nc.scalar.activation(
                                zT_sb[:, t0:t1, :],
                                zpre[:, :(t1 - t0) * 128],
                                AF.Prelu, alpha=0.2)

                        # logits; exp on the small [e, h] logits only
                        lgs = esb.tile([128, CT * 4], BF16, tag="lgs")
                        nsub = math.ceil(CT / 16)
                        for si in range(nsub):
                            t0, t1 = si * 16, min(si * 16 + 16, CT)
                            nt = t1 - t0
                            lg = lgp.tile([128, 64], F32, tag="lg")
                            for t in range(t0, t1):
                                nc.tensor.matmul(
                                    out=lg[:, (t - t0) * 4:(t - t0) * 4 + 4],
                                    lhsT=zT_sb[:, t, :],
                                    rhs=ab[:, layer * H:(layer + 1) * H],
                                    start=True, stop=True)
                            nc.scalar.activation(
                                lgs[:, t0 * 4:t0 * 4 + nt * 4],
                                lg[:, 0:nt * 4], AF.Exp)

                        # mw = xln * w (w broadcast 32-wide via 4D AP), w cols
                        mw = esb.tile([128, CT, 132], BF16, tag="mw")
                        nc.vector.tensor_mul(
                            mw[:, :, 0:128].rearrange(
                                "p t (h c) -> p t h c", h=H),
                            xln[:].rearrange("p t (h c) -> p t h c", h=H),
                            lgs[:].rearrange("p (t h) -> p t h", h=H)
                            [:, :, :, None].to_broadcast([128, CT, H, HID]))
                        nc.vector.tensor_copy(
                            mw[:, :, 128:132],
                            lgs[:].rearrange("p (t h) -> p t h", h=H))

                        accs = [accp.tile([128, 132], F32, tag=f"acc{j}",
                                          name=f"acc{j}")
                                for j in range(C)]
                        for t in range(CT):
                            j, first, last = blk_of(t)
                            nc.tensor.matmul(
                                out=accs[j][:], lhsT=ohs[t],
                                rhs=mw[:, t, :], start=first, stop=last)

                        # epilogue: normalize, +bo, ELU -> hn
                        asb = esb.tile([128, C, 132], F32, tag="asb")
                        for j in range(C):
                            nc.scalar.copy(asb[:, j, :], accs[j][:])
                        rec = esb.tile([128, C * 4], F32, tag="rec")
                        nc.vector.tensor_scalar_max(
                            rec[:].rearrange("p (j h) -> p j h", j=C),
                            asb[:, :, 128:132], 1e-16)
                        nc.vector.reciprocal(rec[:], rec[:])
                        u = esb.tile([128, C * 128], F32, tag="u")
                        nc.vector.tensor_mul(
                            u[:].rearrange("p (j h c) -> p j h c", j=C, h=H),
                            asb[:, :, 0:128].rearrange(
                                "p j (h c) -> p j h c", h=H),
                            rec[:].rearrange("p (j h) -> p j h", j=C)
                            [:, :, :, None].to_broadcast([128, C, H, HID]))
                        nc.vector.tensor_add(
                            u[:].rearrange("p (j f) -> p j f", j=C),
                            u[:].rearrange("p (j f) -> p j f", j=C),
                            bo[:, layer * FH:(layer + 1) * FH]
                            [:, None, :].to_broadcast([128, C, FH]))
                        # elu(u) = (exp(min(u,0)) - 1) + relu(u)
                        r = esb.tile([128, C * 128], F32, tag="r")
                        nc.scalar.activation(r[:], u[:], AF.Relu)
                        tmin = esb.tile([128, C * 128], F32, tag="tmin")
                        nc.vector.tensor_scalar_min(tmin[:], u[:], 0.0)
                        s_ = esb.tile([128, C * 128], F32, tag="s")
                        nc.scalar.activation(s_[:], tmin[:], AF.Exp)
                        hdst = h3f if last_layer else hn
                        nc.vector.scalar_tensor_tensor(
                            out=hdst[:, sc * C * 128:(sc + 1) * C * 128],
                            in0=s_[:], scalar=-1.0, in1=r[:],
                            op0=OP.add, op1=OP.add)

            # ------------- final linear (f32)
            with tc.tile_pool(name="fin", bufs=1) as fin:
                fm = fin.tile([128, NB, 128], F32)
                nc.vector.tensor_mul(
                    fm[:], h3f[:].rearrange("p (b f) -> p b f", b=NB),
                    wfb[:][:, None, :].to_broadcast([128, NB, FH]))
                of = fin.tile([128, NB], F32)
                nc.vector.tensor_reduce(
                    out=of[:], in_=fm[:], axis=mybir.AxisListType.X,
                    op=OP.add)
                nc.vector.tensor_scalar_add(of[:], of[:], bfb[:, 0:1])
                nc.sync.dma_start(out=out_t[:, :], in_=of[:])

    nc.compile()
    return nc


# ---------------------------------------------------------------- inputs


def _to_bf(x):
    return np.asarray(x, np.float32).astype(BF)


def make_inputs(cfg, pre, inputs):
    NB, NPC = cfg.NB, cfg.NPC
    x = np.asarray(inputs["x"], np.float32)
    xp_all = np.zeros((cfg.NTAB, IN), np.float32)
    xp_all[pre["perm_pos"]] = x
    W_l = np.stack([inputs["W_l0"], *[inputs["W_l"][i] for i in range(L - 1)]])
    W_r = np.stack([inputs["W_r0"], *[inputs["W_r"][i] for i in range(L - 1)]])
    att = np.stack([inputs["att0"], *[inputs["att"][i] for i in range(L - 1)]])
    b_l = np.stack([inputs["b_l0"], *[inputs["b_l"][i] for i in range(L - 1)]])
    b_r = np.stack([inputs["b_r0"], *[inputs["b_r"][i] for i in range(L - 1)]])
    bo = np.stack([inputs["bo0"], *[inputs["bo"][i] for i in range(L - 1)]])
    ablk = np.stack([make_ablk(att[l]) for l in range(L)])
    blr = np.stack([b_l, b_r], axis=1).astype(np.float32)
    bo_b = np.repeat(np.asarray(bo, np.float32)[:, None, :], 128, axis=1)
    wf = np.asarray(inputs["W_f"], np.float32)
    wfb = np.repeat(wf[:, 0][None, :], 128, axis=0)
    bfb = np.full((128, 1), float(np.asarray(inputs["b_f"]).ravel()[0]),
                  np.float32)
    iota = np.repeat(np.arange(128, dtype=np.float32)[None, :], 128, axis=0)
    ident = np.eye(128, dtype=np.float32)

    wlr_p = np.concatenate(
        [np.concatenate([W_l[l], W_r[l]], axis=1) for l in range(L)], axis=1)
    ab_p = np.concatenate([ablk[l] for l in range(L)], axis=1)   # [FH, L*H]
    blr_p = blr.reshape(1, -1).astype(np.float32)                # [1, L*2*FH]
    bo_p = np.concatenate([bo_b[l] for l in range(L)], axis=1)   # [128, L*FH]
    shared = dict(
        wlr=_to_bf(wlr_p), ab=_to_bf(ab_p), blr=blr_p,
        bo=bo_p.astype(np.float32), wfb=wfb.astype(np.float32), bfb=bfb,
        iota=_to_bf(iota), ident=_to_bf(ident),
    )
    in_maps = []
    for c in range(NCORES):
        xp = xp_all[c * NPC:(c + 1) * NPC]
        xp_t = np.ascontiguousarray(
            xp.reshape(NB, 128, IN).transpose(1, 0, 2)).reshape(128, NB * IN)
        in_maps.append(dict(
            xp=_to_bf(xp_t),
            ilo=pre["idx_lo"][c], ihi=pre["idx_hi"][c],
            dstl=pre["dstloc"][c],
            **shared,
        ))
    return in_maps


def assemble_output(cfg, pre, out_maps):
    full = np.zeros((cfg.NTAB,), np.float32)
    for c in range(NCORES):
        o = np.asarray(out_maps[c]["out"], np.float32)  # [128, NB]
        full[c * cfg.NPC:(c + 1) * cfg.NPC] = o.T.reshape(-1)
    return full[pre["perm_pos"]][:, None].astype(np.float32)



# ---------------------------------------------------------------- runner


class CompiledSPMD:
    """Compile the bass module once; run it many times on n_cores devices."""

    def __init__(self, nc, n_cores):
        import jax
        from jax.sharding import Mesh, PartitionSpec
        from jax.experimental.shard_map import shard_map
        from concourse import bass2jax
        from concourse.bass2jax import _bass_exec_p, install_neuronx_cc_hook
        self._jax = jax
        install_neuronx_cc_hook()
        self.nc = nc
        self.n_cores = n_cores
        partition_name = (nc.partition_id_tensor.name
                          if nc.partition_id_tensor else None)
        in_names, out_names, out_avals, zero_outs = [], [], [], []
        for alloc in nc.m.functions[0].allocations:
            if not isinstance(alloc, mybir.MemoryLocationSet):
                continue
            name = alloc.memorylocations[0].name
            if alloc.kind == "ExternalInput":
                if name != partition_name and name != (
                        nc.dbg_addr.name if nc.dbg_addr else None):
                    in_names.append(name)
            elif alloc.kind == "ExternalOutput":
                out_names.append(name)
                shape = tuple(alloc.tensor_shape)
                dtype = mybir.dt.np(alloc.dtype)
                out_avals.append(jax.core.ShapedArray(shape, dtype))
                zero_outs.append(np.zeros(shape, dtype))
        self.in_names, self.out_names = in_names, out_names
        self.out_avals, self.zero_outs = out_avals, zero_outs
        n_params, n_outs = len(in_names), len(out_names)
        all_in = list(in_names) + list(out_names)
        if nc.dbg_addr is not None:
            all_in.append(nc.dbg_addr.name)
        if partition_name is not None:
            all_in.append(partition_name)
        dbg_name = nc.dbg_addr.name if nc.dbg_addr is not None else None

        def _body(*args):
            operands = list(args)
            if dbg_name is not None:
                operands.append(jax.numpy.zeros((1, 2), jax.numpy.uint32))
            if partition_name is not None:
                operands.append(bass2jax.partition_id_tensor())
            outs = _bass_exec_p.bind(
                *operands, out_avals=tuple(out_avals),
                in_names=tuple(all_in), out_names=tuple(out_names),
                lowering_input_output_aliases=(),
                sim_require_finite=True, sim_require_nnan=True, nc=nc)
            return tuple(outs)

        devices = jax.devices()[:n_cores]
        assert len(devices) == n_cores
        self._mesh = Mesh(np.asarray(devices), ("core",))
        in_specs = (PartitionSpec("core"),) * (n_params + n_outs)
        out_specs = (PartitionSpec("core"),) * n_outs
        self._P = PartitionSpec
        self._fn = jax.jit(
            shard_map(_body, mesh=self._mesh, in_specs=in_specs,
                      out_specs=out_specs, check_rep=False),
            keep_unused=True)

    def prepare_inputs(self, in_maps):
        jax = self._jax
        assert len(in_maps) == self.n_cores
        concat_in = [
            np.concatenate([np.asarray(in_maps[c][n])
                            for c in range(self.n_cores)], axis=0)
            for n in self.in_names]
        concat_zeros = [
            np.zeros((self.n_cores * z.shape[0], *z.shape[1:]), z.dtype)
            for z in self.zero_outs]
        sh = jax.sharding.NamedSharding(self._mesh, self._P("core"))
        args = [jax.device_put(a, sh) for a in concat_in + concat_zeros]
        jax.block_until_ready(args)
        return args

    def run_to_maps(self, args):
        jax = self._jax
        outs = jax.block_until_ready(self._fn(*args))
        return [
            {name: np.asarray(outs[i]).reshape(
                self.n_cores, *self.out_avals[i].shape)[c]
             for i, name in enumerate(self.out_names)}
            for c in range(self.n_cores)]

    def time_exec(self, args, iters=20, warmup=3):
        import time as _time
        jax = self._jax
        for _ in range(warmup):
            out = self._fn(*args)
        jax.block_until_ready(out)
        t0 = _time.perf_counter()
        outs = [self._fn(*args) for _ in range(iters)]
        jax.block_until_ready(outs)
        return (_time.perf_counter() - t0) / iters


_COMPILED = {}


def kernel(**inputs):
    cfg = FULL
    pre = preprocess(cfg, np.asarray(inputs["edge_index"]))
    key = (cfg.N, pre["TLO"], pre["THI"])
    if key not in _COMPILED:
        nc = build_program(cfg, pre["TLO"], pre["THI"])
        _COMPILED[key] = CompiledSPMD(nc, NCORES)
    comp = _COMPILED[key]
    in_maps = make_inputs(cfg, pre, inputs)
    args = comp.prepare_inputs(in_maps)
    out_maps = comp.run_to_maps(args)
    return assemble_output(cfg, pre, out_maps)



# revision 28
# speedup vs baseline: 2.2629x; 1.0884x over previous
"""GATv2 3-layer GNN on 8 Trainium2 NeuronCores.

Strategy (dst-sharded edge processing, single gather per edge):
- Nodes are bin-packed by in-degree into 8*NB bins of <=128 dst nodes each,
  balancing edges per bin. Bin -> (core, block). The xl feature table is
  stored in permuted order (core-major, block-major, row).
- Per layer, per core: xl/xr for own nodes via PE (transpose + matmul);
  xl shards AllGather'ed into a full table (bf16); xr kept in SBUF.
- Edges (grouped by dst block, padded to uniform tile counts) are processed
  in super-chunks of C blocks: ONE dma_gather of xl rows per edge; xr per
  edge comes from a one-hot-transpose matmul against the dst block's xr
  rows; xl is moved to channel-major via identity-RHS matmuls accumulating
  into the same PSUM (z_pre = xr_expand + xl^T). Leaky-relu on ScalarE,
  logits via PE (ldweights = z^T trick), exp+broadcast on ScalarE,
  messages w*xl on VectorE, segment-sum via one-hot matmuls into PSUM.
- Segment softmax skips max-subtraction (logits are O(1) by construction;
  exact same math, exp is safe in fp32).
- int16 gather indices: edges are split per block into a "lo" group
  (table rows < 32768, base 0) and "hi" group (rows >= BBASE, base BBASE),
  with flexible rows in [BBASE, 32768) used to balance the two groups.
"""
import math
import numpy as np
import ml_dtypes

import concourse.bacc as bacc
import concourse.bass as bass
import concourse.mybir as mybir
import concourse.tile as tile
from concourse.library_config import mlp as mlp_lib


# --- patch Tile's DMASW lane assignment to be SWDGE-queue-aware: a DMA sem
# lane must only ever be updated from one SWDGE queue; Tile round-robins
# lanes obliviously. Pin lanes {2q, 2q+1} to queue q (NQ<=4).
from concourse import tile_sem_assignment as _tsa


def _queue_aware_assign_tick(self, inst, *, _orig=_tsa.TileClockTick._assign_tick):
    if (isinstance(inst, _tsa.DMAInst)
            and inst.engine == mybir.EngineType.Pool):
        q = int(getattr(inst, "queue_num", 0) or 0)
        if not hasattr(self, "_qtog"):
            self._qtog = {}
        t = self._qtog.get(q, 0)
        lanes = max(1, self.swdge_sem_count // 4)
        self.next_sw_dma_idx = (q * lanes + t) % self.swdge_sem_count
        self._qtog[q] = (t + 1) % lanes
    return _orig(self, inst)


_tsa.TileClockTick._assign_tick = _queue_aware_assign_tick

F32 = mybir.dt.float32
BF16 = mybir.dt.bfloat16
I16 = mybir.dt.int16
BF = ml_dtypes.bfloat16
AF = mybir.ActivationFunctionType
OP = mybir.AluOpType

NCORES = 8
IN, HID, H, OUT, NLAYERS = 128, 32, 4, 1, 3
FH = HID * H  # 128
L = NLAYERS


class Cfg:
    def __init__(self, N, E_raw, NB, C=2, SPLIT=32768, NQ=4):
        self.N = N
        self.NB = NB            # blocks (bins) per core
        self.C = C              # blocks per super-chunk
        assert NB % C == 0
        self.NSC = NB // C
        self.NPC = NB * 128     # table rows per core
        self.NTAB = NCORES * self.NPC
        self.SPLIT = min(SPLIT, self.NTAB)
        self.BBASE = max(0, self.NTAB - self.SPLIT)  # hi-group table base
        self.NQ = NQ
        assert self.NTAB - self.BBASE <= self.SPLIT


FULL = Cfg(N=50000, E_raw=800000, NB=50)


# ---------------------------------------------------------------- host side


def _wrap_idx16(idx, pad_to):
    """[n] ints -> [128, pad_to//16] int16 dma_gather index layout
    (i at partition i%16, col i//16; replicated into all 8 groups)."""
    a = np.zeros(pad_to, np.int64)
    a[: len(idx)] = idx
    w = a.reshape(pad_to // 16, 16).T.astype(np.int16)
    return np.tile(w, (8, 1))


def preprocess(cfg, edge_index):
    N, NB, C = cfg.N, cfg.NB, cfg.C
    nbins = NCORES * NB
    src = np.concatenate([np.asarray(edge_index[0], np.int64),
                          np.arange(N, dtype=np.int64)])
    dst = np.concatenate([np.asarray(edge_index[1], np.int64),
                          np.arange(N, dtype=np.int64)])
    deg = np.bincount(dst, minlength=N)

    # snake-deal nodes (sorted by in-degree desc) into bins
    order = np.argsort(-deg, kind="stable")
    nrounds = math.ceil(N / nbins)
    binof = np.empty(N, np.int64)
    rowof = np.empty(N, np.int64)
    for r in range(nrounds):
        chunk = order[r * nbins:(r + 1) * nbins]
        cols = np.arange(len(chunk))
        if r % 2:
            cols = nbins - 1 - cols
        binof[chunk] = cols
        rowof[chunk] = r
    assert rowof.max() <= 127
    perm_pos = binof * 128 + rowof          # node -> table row

    psrc = perm_pos[src]
    ebin = binof[dst]
    erow = rowof[dst]

    eorder = np.argsort(ebin, kind="stable")
    psrc = psrc[eorder]
    erow = erow[eorder]
    counts = np.bincount(ebin[eorder], minlength=nbins)
    starts = np.concatenate([[0], np.cumsum(counts)])

    # lo/hi split with flexible band
    grp = np.zeros(len(psrc), np.int8)
    nlo = np.zeros(nbins, np.int64)
    for b in range(nbins):
        s, e = starts[b], starts[b + 1]
        p = psrc[s:e]
        lofix = p < cfg.BBASE
        hifix = p >= cfg.SPLIT
        flex = ~lofix & ~hifix
        a, bb, f = int(lofix.sum()), int(hifix.sum()), int(flex.sum())
        x = int(np.clip((bb + f - a + 1) // 2, 0, f))
        g = np.zeros(e - s, np.int8)
        g[hifix] = 1
        fi = np.nonzero(flex)[0]
        g[fi[x:]] = 1
        grp[s:e] = g
        nlo[b] = a + x
    nhi = counts - nlo
    if cfg.NTAB <= cfg.SPLIT:
        grp[:] = 0
        nlo = counts.copy()
        nhi[:] = 0
    TLO = max(1, int(np.ceil(nlo.max() / 128)))
    THI = int(np.ceil(nhi.max() / 128))
    TT = TLO + THI

    idx_lo = np.zeros((NCORES, cfg.NSC, 128, C * TLO * 128 // 16), np.int16)
    idx_hi = np.zeros((NCORES, cfg.NSC, 128, max(1, C * THI * 128 // 16)),
                      np.int16)
    dstloc = np.full((NCORES, cfg.NSC, C * TT, 128), 255.0, np.float32)

    for core in range(NCORES):
        for sc in range(cfg.NSC):
            blocks = [core * NB + sc * C + j for j in range(C)]
            lo_list, hi_list = [], []
            for j, b in enumerate(blocks):
                s, e = starts[b], starts[b + 1]
                g = grp[s:e]
                p = psrc[s:e].copy()
                r = erow[s:e]
                for gi, (tbase, tcnt, lst) in enumerate(
                    ((0, TLO, lo_list), (C * TLO, THI, hi_list))
                ):
                    sel = g == gi
                    pp = p[sel]
                    rr = r[sel]
                    if gi == 1:
                        pp = pp - cfg.BBASE
                    assert len(pp) <= tcnt * 128
                    pad = tcnt * 128 - len(pp)
                    ppad = np.concatenate([pp, np.zeros(pad, np.int64)])
                    dpad = np.concatenate(
                        [rr.astype(np.float32),
                         np.full(pad, 255.0, np.float32)])
                    lst.append(ppad)
                    for t in range(tcnt):
                        gt = tbase + j * tcnt + t
                        dstloc[core, sc, gt] = dpad[t * 128:(t + 1) * 128]
            idx_lo[core, sc] = _wrap_idx16(np.concatenate(lo_list),
                                           C * TLO * 128)
            if THI:
                idx_hi[core, sc] = _wrap_idx16(np.concatenate(hi_list),
                                               C * THI * 128)

    # dstloc -> [128 edge-row, C*TT] per (core, sc)
    dstloc = np.ascontiguousarray(dstloc.transpose(0, 1, 3, 2))
    return dict(perm_pos=perm_pos, TLO=TLO, THI=THI,
                idx_lo=idx_lo, idx_hi=idx_hi, dstloc=dstloc.astype(BF))


def make_ablk(att_l):  # [H, HID] -> [FH, H]
    a = np.zeros((FH, H), np.float32)
    for h in range(H):
        a[h * HID:(h + 1) * HID, h] = att_l[h]
    return a


# ---------------------------------------------------------------- program


def build_program(cfg, TLO, THI, reps=1, ablate="none"):
    nc = bacc.Bacc("TRN2", target_bir_lowering=False, debug=False,
                   num_devices=NCORES, num_swdge_queues=cfg.NQ)
    NB, C, NSC, NPC, NTAB = cfg.NB, cfg.C, cfg.NSC, cfg.NPC, cfg.NTAB
    TT = TLO + THI
    CT = C * TT
    CE = CT * 128

    x_in = nc.dram_tensor("xp", [128, NPC], BF16, kind="ExternalInput")
    ilo = nc.dram_tensor("ilo", [NSC, 128, C * TLO * 128 // 16], I16,
                         kind="ExternalInput")
    ihi = nc.dram_tensor("ihi", [NSC, 128, max(1, C * THI * 128 // 16)], I16,
                         kind="ExternalInput")
    dstl = nc.dram_tensor("dstl", [NSC, 128, CT], BF16, kind="ExternalInput")
    wlr_in = nc.dram_tensor("wlr", [FH, L * 2 * FH], BF16,
                            kind="ExternalInput")
    ab_in = nc.dram_tensor("ab", [FH, L * H], BF16, kind="ExternalInput")
    blr_in = nc.dram_tensor("blr", [1, L * 2 * FH], F32, kind="ExternalInput")
    blrb_in = nc.dram_tensor("blrb", [128, L * 2 * FH], F32,
                             kind="ExternalInput")
    bo_in = nc.dram_tensor("bo", [128, L * FH], F32, kind="ExternalInput")
    wf_in = nc.dram_tensor("wfb", [128, FH], F32, kind="ExternalInput")
    bf_in = nc.dram_tensor("bfb", [128, 1], F32, kind="ExternalInput")
    iota_in = nc.dram_tensor("iota", [128, 128], BF16, kind="ExternalInput")
    id_in = nc.dram_tensor("ident", [128, 128], BF16, kind="ExternalInput")
    out_t = nc.dram_tensor("out", [128, NB], F32, kind="ExternalOutput")

    with tile.TileContext(nc) as tc:
        with (
            tc.tile_pool(name="const", bufs=1) as cpool,
            tc.tile_pool(name="big", bufs=1) as bigp,
            tc.tile_pool(name="dram", bufs=1, space="DRAM") as dram,
        ):
            nc.gpsimd.load_library(mlp_lib)
            iota = cpool.tile([128, 128], BF16)
            nc.sync.dma_start(out=iota[:], in_=iota_in[:, :])
            ident = cpool.tile([128, 128], BF16)
            nc.sync.dma_start(out=ident[:], in_=id_in[:, :])
            wlr = cpool.tile([FH, L * 2 * FH], BF16)
            nc.sync.dma_start(out=wlr[:], in_=wlr_in[:, :])
            ab = cpool.tile([FH, L * H], BF16)
            nc.sync.dma_start(out=ab[:], in_=ab_in[:, :])
            blr = cpool.tile([1, L * 2 * FH], F32)
            nc.sync.dma_start(out=blr[:], in_=blr_in[:, :])
            blrb = cpool.tile([128, L * 2 * FH], F32)
            nc.sync.dma_start(out=blrb[:], in_=blrb_in[:, :])
            bo = cpool.tile([128, L * FH], F32)
            nc.sync.dma_start(out=bo[:], in_=bo_in[:, :])
            wfb = cpool.tile([128, FH], F32)
            nc.sync.dma_start(out=wfb[:], in_=wf_in[:, :])
            bfb = cpool.tile([128, 1], F32)
            nc.sync.dma_start(out=bfb[:], in_=bf_in[:, :])
            ones1 = cpool.tile([1, 128], F32)
            nc.vector.memset(ones1[:], 1.0)

            hbuf = [bigp.tile([128, NPC], BF16, tag=f"h{i}", name=f"h{i}")
                    for i in range(2)]
            xr_pin = bigp.tile([128, NPC], BF16, tag="xrp")
            nc.gpsimd.dma_start(out=hbuf[0][:], in_=x_in[:, :])

            xl_shards = [dram.tile([NPC, FH], BF16, name=f"xl_shard{i}")
                         for i in range(L * reps)]
            xl_fulls = [dram.tile([NTAB, FH], BF16, addr_space="Shared",
                                  name=f"xl_full{i}") for i in range(L * reps)]
            h3f = bigp.tile([128, NPC], F32, tag="h3f")

            for rep in range(reps):
              if rep > 0:
                nc.gpsimd.dma_start(out=hbuf[0][:], in_=x_in[:, :])
              for layer in range(L):
                h = hbuf[layer % 2]
                hn = hbuf[(layer + 1) % 2]
                last_layer = layer == L - 1
                xl_shard = xl_shards[rep * L + layer]
                xl_full = xl_fulls[rep * L + layer]
                # ------------- phase M (batched: 4 blocks per group)
                with (
                    tc.tile_pool(name=f"mp{layer}", bufs=2, space="PSUM") as mp,
                    tc.tile_pool(name=f"mx{layer}", bufs=2, space="PSUM") as mx,
                    tc.tile_pool(name=f"ms{layer}", bufs=3) as msb,
                ):
                    for b0 in range(0, NB, 4):
                        nbk = min(4, NB - b0)
                        hT_ps = mp.tile([128, 4, 128], BF16, tag="hT")
                        for j in range(nbk):
                            b = b0 + j
                            nc.tensor.transpose(
                                out=hT_ps[:, j, :],
                                in_=h[:, b * 128:(b + 1) * 128],
                                identity=ident[:])
                        hT = msb.tile([128, 4, 128], BF16, tag="hT")
                        nc.scalar.copy(hT[:, 0:nbk, :], hT_ps[:, 0:nbk, :])
                        ps = [mx.tile([128, 2, 256], F32, tag=f"x{i}",
                                      name=f"x{i}") for i in range(2)]
                        for j in range(nbk):
                            nc.tensor.matmul(
                                out=ps[j // 2][:, j % 2, :],
                                lhsT=hT[:, j, :],
                                rhs=wlr[:, layer * 2 * FH:
                                        (layer + 1) * 2 * FH],
                                start=True, stop=True)
                        xl_sb = msb.tile([128, 4, 128], BF16, tag="xlsb")
                        for i in range((nbk + 1) // 2):
                            nj = min(2, nbk - 2 * i)
                            nc.vector.tensor_add(
                                xl_sb[:, 2 * i:2 * i + nj, :],
                                ps[i][:, 0:nj, 0:128],
                                blrb[:, layer * 2 * FH:
                                     layer * 2 * FH + 128]
                                [:, None, :].to_broadcast([128, nj, 128]))
                            nc.vector.tensor_add(
                                xr_pin[:, (b0 + 2 * i) * 128:
                                       (b0 + 2 * i + nj) * 128]
                                .rearrange("p (j c) -> p j c", j=nj),
                                ps[i][:, 0:nj, 128:256],
                                blrb[:, layer * 2 * FH + 128:
                                     layer * 2 * FH + 256]
                                [:, None, :].to_broadcast([128, nj, 128]))
                        for j in range(nbk):
                            nc.sync.dma_start(
                                out=xl_shard[(b0 + j) * 128:
                                             (b0 + j + 1) * 128, :],
                                in_=xl_sb[:, j, :])

                nc.gpsimd.collective_compute(
                    "AllGather", OP.bypass,
                    replica_groups=[list(range(NCORES))],
                    ins=[xl_shard.opt()], outs=[xl_full.opt()],
                )

                # ------------- phase E
                with (
                    tc.tile_pool(name=f"ea{layer}", bufs=1, space="PSUM") as accp,
                    tc.tile_pool(name=f"ez{layer}", bufs=2, space="PSUM") as zp,
                    tc.tile_pool(name=f"eo{layer}", bufs=2, space="PSUM") as otp,
                    tc.tile_pool(name=f"el{layer}", bufs=2, space="PSUM") as lgp,
                    tc.tile_pool(name=f"es{layer}", bufs=2) as esb,
                    tc.tile_pool(name=f"eoh{layer}", bufs=2) as ohp,
                ):
                    for sc in range(NSC):
                        itlo = esb.tile([128, C * TLO * 128 // 16], I16,
                                        tag="itlo")
                        nc.sync.dma_start(out=itlo[:], in_=ilo[sc])
                        if THI:
                            ithi = esb.tile([128, C * THI * 128 // 16], I16,
                                            tag="ithi")
                            nc.sync.dma_start(out=ithi[:], in_=ihi[sc])
                        dl = esb.tile([128, CT], BF16, tag="dl")
                        nc.sync.dma_start(out=dl[:], in_=dstl[sc])

                        xln = esb.tile([128, CT, 128], BF16, tag="xln")
                        nlo_e = C * TLO * 128
                        if ablate == "nogather":
                            nc.vector.memset(xln[:, 0:1, :], 0.5)
                        else:
                            qb = 0
                            for base, ntile, it in (
                                (0, C * TLO, itlo),
                                (C * TLO, C * THI, ithi if THI else None),
                            ):
                                if not ntile:
                                    continue
                                srcap = (xl_full[:, :] if base == 0
                                         else xl_full[cfg.BBASE:, :])
                                half = ntile // 2
                                parts = ([(0, half), (half, ntile - half)]
                                         if half else [(0, ntile)])
                                for (o, n) in parts:
                                    ne = n * 128
                                    # idx slice: 8 columns per tile
                                    nc.gpsimd.dma_gather(
                                        out_ap=xln[:, base + o:base + o + n, :],
                                        in_ap=srcap,
                                        idxs_ap=it[:, o * 8:(o + n) * 8],
                                        num_idxs=ne, num_idxs_reg=ne,
                                        elem_size=FH,
                                        queue_num=(4 * sc + qb) % cfg.NQ,
                                        single_packet=False)
                                    qb += 1

                        if ablate == "nocompute":
                            nc.vector.memset(
                                hn[:, sc * C * 128:(sc + 1) * C * 128], 0.01)
                            if last_layer:
                                nc.vector.memset(
                                    h3f[:, sc * C * 128:(sc + 1) * C * 128],
                                    0.01)
                            continue

                        def blk_of(t):
                            if t < C * TLO:
                                j = t // TLO
                                first = (t % TLO) == 0
                                last = THI == 0 and (t % TLO) == TLO - 1
                            else:
                                j = (t - C * TLO) // THI
                                first = False
                                last = ((t - C * TLO) % THI) == THI - 1
                            return j, first, last

                        # one-hots (4 batched DVE builds) + transposes
                        OHB = 9
                        assert CT % OHB == 0
                        oh_all = ohp.tile([128, CT, 128], BF16, tag="oh")
                        for g in range(CT // OHB):
                            nc.vector.tensor_tensor(
                                out=oh_all[:, g * OHB:(g + 1) * OHB, :],
                                in0=iota[:][:, None, :].to_broadcast(
                                    [128, OHB, 128]),
                                in1=dl[:, g * OHB:(g + 1) * OHB, None]
                                .to_broadcast([128, OHB, 128]),
                                op=OP.is_equal)
                        ohs = [oh_all[:, t, :] for t in range(CT)]
                        n8 = math.ceil(CT / 8)
                        ohT_sb = esb.tile([128, CT, 128], BF16, tag="ohT")
                        for g in range(n8):
                            t0, t1 = g * 8, min(g * 8 + 8, CT)
                            ohT_ps = otp.tile([128, 8 * 128], BF16, tag="ohT")
                            for t in range(t0, t1):
                                nc.tensor.transpose(
                                    out=ohT_ps[:, (t - t0) * 128:
                                               (t - t0 + 1) * 128],
                                    in_=ohs[t], identity=ident[:])
                            nc.scalar.copy(
                                ohT_sb[:, t0:t1, :],
                                ohT_ps[:, :(t1 - t0) * 128])

                        # z_pre = xr_expand + xl^T ; leaky; z^T in SBUF
                        zT_sb = esb.tile([128, CT, 128], BF16, tag="zT")
                        n4 = math.ceil(CT / 4)
                        for g in range(n4):
                            t0, t1 = g * 4, min(g * 4 + 4, CT)
                            zpre = zp.tile([128, 4 * 128], F32, tag="zpre")
                            for t in range(t0, t1):
                                j, _, _ = blk_of(t)
                                bcol = (sc * C + j) * 128
                                o = (t - t0) * 128
                                nc.tensor.matmul(
                                    out=zpre[:, o:o + 128],
                                    lhsT=xr_pin[:, bcol:bcol + 128],
                                    rhs=ohT_sb[:, t, :],
                                    start=True, stop=False)
                                nc.tensor.matmul(
                                    out=zpre[:, o:o + 128],
                                    lhsT=xln[:, t, :], rhs=ident[:],
                                    start=False, stop=True)
                            nc.scalar.activation(
                                zT_sb[:, t0:t1, :],
                                zpre[:, :(t1 - t0) * 128],
                                AF.Prelu, alpha=0.2)

                        # logits; exp on the small [e, h] logits only
                        lgs = esb.tile([128, CT * 4], BF16, tag="lgs")
                        nsub = math.ceil(CT / 16)
                        for si in range(nsub):
                            t0, t1 = si * 16, min(si * 16 + 16, CT)
                            nt = t1 - t0
                            lg = lgp.tile([128, 64], F32, tag="lg")
                            for t in range(t0, t1):
                                nc.tensor.matmul(
                                    out=lg[:, (t - t0) * 4:(t - t0) * 4 + 4],
                                    lhsT=zT_sb[:, t, :],
                                    rhs=ab[:, layer * H:(layer + 1) * H],
                                    start=True, stop=True)
                            nc.scalar.activation(
                                lgs[:, t0 * 4:t0 * 4 + nt * 4],
                                lg[:, 0:nt * 4], AF.Exp)

                        # mw = xln * w (w broadcast 32-wide via 4D AP), w cols
                        mw = esb.tile([128, CT, 132], BF16, tag="mw")
                        nc.vector.tensor_mul(
                            mw[:, :, 0:128].rearrange(
                                "p t (h c) -> p t h c", h=H),
                            xln[:].rearrange("p t (h c) -> p t h c", h=H),
                            lgs[:].rearrange("p (t h) -> p t h", h=H)
                            [:, :, :, None].to_broadcast([128, CT, H, HID]))
                        nc.vector.tensor_copy(
                            mw[:, :, 128:132],
                            lgs[:].rearrange("p (t h) -> p t h", h=H))

                        accs = [accp.tile([128, 132], F32, tag=f"acc{j}",
                                          name=f"acc{j}")
                                for j in range(C)]
                        for t in range(CT):
                            j, first, last = blk_of(t)
                            nc.tensor.matmul(
                                out=accs[j][:], lhsT=ohs[t],
                                rhs=mw[:, t, :], start=first, stop=last)

                        # epilogue: normalize, +bo, ELU -> hn
                        asb = esb.tile([128, C, 132], F32, tag="asb")
                        for j in range(C):
                            nc.scalar.copy(asb[:, j, :], accs[j][:])
                        rec = esb.tile([128, C * 4], F32, tag="rec")
                        nc.vector.tensor_scalar_max(
                            rec[:].rearrange("p (j h) -> p j h", j=C),
                            asb[:, :, 128:132], 1e-16)
                        nc.vector.reciprocal(rec[:], rec[:])
                        u = esb.tile([128, C * 128], F32, tag="u")
                        nc.vector.tensor_mul(
                            u[:].rearrange("p (j h c) -> p j h c", j=C, h=H),
                            asb[:, :, 0:128].rearrange(
                                "p j (h c) -> p j h c", h=H),
                            rec[:].rearrange("p (j h) -> p j h", j=C)
                            [:, :, :, None].to_broadcast([128, C, H, HID]))
                        nc.vector.tensor_add(
                            u[:].rearrange("p (j f) -> p j f", j=C),
                            u[:].rearrange("p (j f) -> p j f", j=C),
                            bo[:, layer * FH:(layer + 1) * FH]
                            [:, None, :].to_broadcast([128, C, FH]))
                        # elu(u) = (exp(min(u,0)) - 1) + relu(u)
                        r = esb.tile([128, C * 128], F32, tag="r")
                        nc.scalar.activation(r[:], u[:], AF.Relu)
                        tmin = esb.tile([128, C * 128], F32, tag="tmin")
                        nc.vector.tensor_scalar_min(tmin[:], u[:], 0.0)
                        s_ = esb.tile([128, C * 128], F32, tag="s")
                        nc.scalar.activation(s_[:], tmin[:], AF.Exp)
                        hdst = h3f if last_layer else hn
                        nc.vector.scalar_tensor_tensor(
                            out=hdst[:, sc * C * 128:(sc + 1) * C * 128],
                            in0=s_[:], scalar=-1.0, in1=r[:],
                            op0=OP.add, op1=OP.add)

            # ------------- final linear (f32)
            with tc.tile_pool(name="fin", bufs=1) as fin:
                fm = fin.tile([128, NB, 128], F32)
                nc.vector.tensor_mul(
                    fm[:], h3f[:].rearrange("p (b f) -> p b f", b=NB),
                    wfb[:][:, None, :].to_broadcast([128, NB, FH]))
                of = fin.tile([128, NB], F32)
                nc.vector.tensor_reduce(
                    out=of[:], in_=fm[:], axis=mybir.AxisListType.X,
                    op=OP.add)
                nc.vector.tensor_scalar_add(of[:], of[:], bfb[:, 0:1])
                nc.sync.dma_start(out=out_t[:, :], in_=of[:])

    nc.compile()
    return nc


# ---------------------------------------------------------------- inputs


def _to_bf(x):
    return np.asarray(x, np.float32).astype(BF)


def make_inputs(cfg, pre, inputs):
    NB, NPC = cfg.NB, cfg.NPC
    x = np.asarray(inputs["x"], np.float32)
    xp_all = np.zeros((cfg.NTAB, IN), np.float32)
    xp_all[pre["perm_pos"]] = x
    W_l = np.stack([inputs["W_l0"], *[inputs["W_l"][i] for i in range(L - 1)]])
    W_r = np.stack([inputs["W_r0"], *[inputs["W_r"][i] for i in range(L - 1)]])
    att = np.stack([inputs["att0"], *[inputs["att"][i] for i in range(L - 1)]])
    b_l = np.stack([inputs["b_l0"], *[inputs["b_l"][i] for i in range(L - 1)]])
    b_r = np.stack([inputs["b_r0"], *[inputs["b_r"][i] for i in range(L - 1)]])
    bo = np.stack([inputs["bo0"], *[inputs["bo"][i] for i in range(L - 1)]])
    ablk = np.stack([make_ablk(att[l]) for l in range(L)])
    blr = np.stack([b_l, b_r], axis=1).astype(np.float32)
    bo_b = np.repeat(np.asarray(bo, np.float32)[:, None, :], 128, axis=1)
    wf = np.asarray(inputs["W_f"], np.float32)
    wfb = np.repeat(wf[:, 0][None, :], 128, axis=0)
    bfb = np.full((128, 1), float(np.asarray(inputs["b_f"]).ravel()[0]),
                  np.float32)
    iota = np.repeat(np.arange(128, dtype=np.float32)[None, :], 128, axis=0)
    ident = np.eye(128, dtype=np.float32)

    wlr_p = np.concatenate(
        [np.concatenate([W_l[l], W_r[l]], axis=1) for l in range(L)], axis=1)
    ab_p = np.concatenate([ablk[l] for l in range(L)], axis=1)   # [FH, L*H]
    blr_p = blr.reshape(1, -1).astype(np.float32)                # [1, L*2*FH]
    bo_p = np.concatenate([bo_b[l] for l in range(L)], axis=1)   # [128, L*FH]
    blrb_p = np.repeat(blr_p, 128, axis=0)
    shared = dict(
        wlr=_to_bf(wlr_p), ab=_to_bf(ab_p), blr=blr_p,
        blrb=blrb_p.astype(np.float32),
        bo=bo_p.astype(np.float32), wfb=wfb.astype(np.float32), bfb=bfb,
        iota=_to_bf(iota), ident=_to_bf(ident),
    )
    in_maps = []
    for c in range(NCORES):
        xp = xp_all[c * NPC:(c + 1) * NPC]
        xp_t = np.ascontiguousarray(
            xp.reshape(NB, 128, IN).transpose(1, 0, 2)).reshape(128, NB * IN)
        in_maps.append(dict(
            xp=_to_bf(xp_t),
            ilo=pre["idx_lo"][c], ihi=pre["idx_hi"][c],
            dstl=pre["dstloc"][c],
            **shared,
        ))
    return in_maps


def assemble_output(cfg, pre, out_maps):
    full = np.zeros((cfg.NTAB,), np.float32)
    for c in range(NCORES):
        o = np.asarray(out_maps[c]["out"], np.float32)  # [128, NB]
        full[c * cfg.NPC:(c + 1) * cfg.NPC] = o.T.reshape(-1)
    return full[pre["perm_pos"]][:, None].astype(np.float32)



# ---------------------------------------------------------------- runner


class CompiledSPMD:
    """Compile the bass module once; run it many times on n_cores devices."""

    def __init__(self, nc, n_cores):
        import jax
        from jax.sharding import Mesh, PartitionSpec
        from jax.experimental.shard_map import shard_map
        from concourse import bass2jax
        from concourse.bass2jax import _bass_exec_p, install_neuronx_cc_hook
        self._jax = jax
        install_neuronx_cc_hook()
        self.nc = nc
        self.n_cores = n_cores
        partition_name = (nc.partition_id_tensor.name
                          if nc.partition_id_tensor else None)
        in_names, out_names, out_avals, zero_outs = [], [], [], []
        for alloc in nc.m.functions[0].allocations:
            if not isinstance(alloc, mybir.MemoryLocationSet):
                continue
            name = alloc.memorylocations[0].name
            if alloc.kind == "ExternalInput":
                if name != partition_name and name != (
                        nc.dbg_addr.name if nc.dbg_addr else None):
                    in_names.append(name)
            elif alloc.kind == "ExternalOutput":
                out_names.append(name)
                shape = tuple(alloc.tensor_shape)
                dtype = mybir.dt.np(alloc.dtype)
                out_avals.append(jax.core.ShapedArray(shape, dtype))
                zero_outs.append(np.zeros(shape, dtype))
        self.in_names, self.out_names = in_names, out_names
        self.out_avals, self.zero_outs = out_avals, zero_outs
        n_params, n_outs = len(in_names), len(out_names)
        all_in = list(in_names) + list(out_names)
        if nc.dbg_addr is not None:
            all_in.append(nc.dbg_addr.name)
        if partition_name is not None:
            all_in.append(partition_name)
        dbg_name = nc.dbg_addr.name if nc.dbg_addr is not None else None

        def _body(*args):
            operands = list(args)
            if dbg_name is not None:
                operands.append(jax.numpy.zeros((1, 2), jax.numpy.uint32))
            if partition_name is not None:
                operands.append(bass2jax.partition_id_tensor())
            outs = _bass_exec_p.bind(
                *operands, out_avals=tuple(out_avals),
                in_names=tuple(all_in), out_names=tuple(out_names),
                lowering_input_output_aliases=(),
                sim_require_finite=True, sim_require_nnan=True, nc=nc)
            return tuple(outs)

        devices = jax.devices()[:n_cores]
        assert len(devices) == n_cores
        self._mesh = Mesh(np.asarray(devices), ("core",))
        in_specs = (PartitionSpec("core"),) * (n_params + n_outs)
        out_specs = (PartitionSpec("core"),) * n_outs
        self._P = PartitionSpec
        self._fn = jax.jit(
            shard_map(_body, mesh=self._mesh, in_specs=in_specs,
                      out_specs=out_specs, check_rep=False),
            keep_unused=True)

    def prepare_inputs(self, in_maps):
        jax = self._jax
        assert len(in_maps) == self.n_cores
        concat_in = [
            np.concatenate([np.asarray(in_maps[c][n])
                            for c in range(self.n_cores)], axis=0)
            for n in self.in_names]
        concat_zeros = [
            np.zeros((self.n_cores * z.shape[0], *z.shape[1:]), z.dtype)
            for z in self.zero_outs]
        sh = jax.sharding.NamedSharding(self._mesh, self._P("core"))
        args = [jax.device_put(a, sh) for a in concat_in + concat_zeros]
        jax.block_until_ready(args)
        return args

    def run_to_maps(self, args):
        jax = self._jax
        outs = jax.block_until_ready(self._fn(*args))
        return [
            {name: np.asarray(outs[i]).reshape(
                self.n_cores, *self.out_avals[i].shape)[c]
             for i, name in enumerate(self.out_names)}
            for c in range(self.n_cores)]

    def time_exec(self, args, iters=20, warmup=3):
        import time as _time
        jax = self._jax
        for _ in range(warmup):
            out = self._fn(*args)
        jax.block_until_ready(out)
        t0 = _time.perf_counter()
        outs = [self._fn(*args) for _ in range(iters)]
        jax.block_until_ready(outs)
        return (_time.perf_counter() - t0) / iters


_COMPILED = {}


def kernel(**inputs):
    cfg = FULL
    pre = preprocess(cfg, np.asarray(inputs["edge_index"]))
    key = (cfg.N, pre["TLO"], pre["THI"])
    if key not in _COMPILED:
        nc = build_program(cfg, pre["TLO"], pre["THI"])
        _COMPILED[key] = CompiledSPMD(nc, NCORES)
    comp = _COMPILED[key]
    in_maps = make_inputs(cfg, pre, inputs)
    args = comp.prepare_inputs(in_maps)
    out_maps = comp.run_to_maps(args)
    return assemble_output(cfg, pre, out_maps)



# revision 30
# speedup vs baseline: 2.2746x; 1.0052x over previous
"""GATv2 3-layer GNN on 8 Trainium2 NeuronCores.

Strategy (dst-sharded edge processing, single gather per edge):
- Nodes are bin-packed by in-degree into 8*NB bins of <=128 dst nodes each,
  balancing edges per bin. Bin -> (core, block). The xl feature table is
  stored in permuted order (core-major, block-major, row).
- Per layer, per core: xl/xr for own nodes via PE (transpose + matmul);
  xl shards AllGather'ed into a full table (bf16); xr kept in SBUF.
- Edges (grouped by dst block, padded to uniform tile counts) are processed
  in super-chunks of C blocks: ONE dma_gather of xl rows per edge; xr per
  edge comes from a one-hot-transpose matmul against the dst block's xr
  rows; xl is moved to channel-major via identity-RHS matmuls accumulating
  into the same PSUM (z_pre = xr_expand + xl^T). Leaky-relu on ScalarE,
  logits via PE (ldweights = z^T trick), exp+broadcast on ScalarE,
  messages w*xl on VectorE, segment-sum via one-hot matmuls into PSUM.
- Segment softmax skips max-subtraction (logits are O(1) by construction;
  exact same math, exp is safe in fp32).
- int16 gather indices: edges are split per block into a "lo" group
  (table rows < 32768, base 0) and "hi" group (rows >= BBASE, base BBASE),
  with flexible rows in [BBASE, 32768) used to balance the two groups.
"""
import math
import numpy as np
import ml_dtypes

import concourse.bacc as bacc
import concourse.bass as bass
import concourse.mybir as mybir
import concourse.tile as tile
from concourse.library_config import mlp as mlp_lib


# --- patch Tile's DMASW lane assignment to be SWDGE-queue-aware: a DMA sem
# lane must only ever be updated from one SWDGE queue; Tile round-robins
# lanes obliviously. Pin lanes {2q, 2q+1} to queue q (NQ<=4).
from concourse import tile_sem_assignment as _tsa


def _queue_aware_assign_tick(self, inst, *, _orig=_tsa.TileClockTick._assign_tick):
    if (isinstance(inst, _tsa.DMAInst)
            and inst.engine == mybir.EngineType.Pool):
        q = int(getattr(inst, "queue_num", 0) or 0)
        if not hasattr(self, "_qtog"):
            self._qtog = {}
        t = self._qtog.get(q, 0)
        lanes = max(1, self.swdge_sem_count // 4)
        self.next_sw_dma_idx = (q * lanes + t) % self.swdge_sem_count
        self._qtog[q] = (t + 1) % lanes
    return _orig(self, inst)


_tsa.TileClockTick._assign_tick = _queue_aware_assign_tick

F32 = mybir.dt.float32
BF16 = mybir.dt.bfloat16
I16 = mybir.dt.int16
BF = ml_dtypes.bfloat16
AF = mybir.ActivationFunctionType
OP = mybir.AluOpType

NCORES = 8
IN, HID, H, OUT, NLAYERS = 128, 32, 4, 1, 3
FH = HID * H  # 128
L = NLAYERS


class Cfg:
    def __init__(self, N, E_raw, NB, C=2, SPLIT=32768, NQ=4):
        self.N = N
        self.NB = NB            # blocks (bins) per core
        self.C = C              # blocks per super-chunk
        assert NB % C == 0
        self.NSC = NB // C
        self.NPC = NB * 128     # table rows per core
        self.NTAB = NCORES * self.NPC
        self.SPLIT = min(SPLIT, self.NTAB)
        self.BBASE = max(0, self.NTAB - self.SPLIT)  # hi-group table base
        self.NQ = NQ
        assert self.NTAB - self.BBASE <= self.SPLIT


FULL = Cfg(N=50000, E_raw=800000, NB=50)


# ---------------------------------------------------------------- host side


def _wrap_idx16(idx, pad_to):
    """[n] ints -> [128, pad_to//16] int16 dma_gather index layout
    (i at partition i%16, col i//16; replicated into all 8 groups)."""
    a = np.zeros(pad_to, np.int64)
    a[: len(idx)] = idx
    w = a.reshape(pad_to // 16, 16).T.astype(np.int16)
    return np.tile(w, (8, 1))


def preprocess(cfg, edge_index):
    N, NB, C = cfg.N, cfg.NB, cfg.C
    nbins = NCORES * NB
    src = np.concatenate([np.asarray(edge_index[0], np.int64),
                          np.arange(N, dtype=np.int64)])
    dst = np.concatenate([np.asarray(edge_index[1], np.int64),
                          np.arange(N, dtype=np.int64)])
    deg = np.bincount(dst, minlength=N)

    # snake-deal nodes (sorted by in-degree desc) into bins
    order = np.argsort(-deg, kind="stable")
    nrounds = math.ceil(N / nbins)
    binof = np.empty(N, np.int64)
    rowof = np.empty(N, np.int64)
    for r in range(nrounds):
        chunk = order[r * nbins:(r + 1) * nbins]
        cols = np.arange(len(chunk))
        if r % 2:
            cols = nbins - 1 - cols
        binof[chunk] = cols
        rowof[chunk] = r
    assert rowof.max() <= 127
    perm_pos = binof * 128 + rowof          # node -> table row

    psrc = perm_pos[src]
    ebin = binof[dst]
    erow = rowof[dst]

    eorder = np.argsort(ebin, kind="stable")
    psrc = psrc[eorder]
    erow = erow[eorder]
    counts = np.bincount(ebin[eorder], minlength=nbins)
    starts = np.concatenate([[0], np.cumsum(counts)])

    # lo/hi split with flexible band
    grp = np.zeros(len(psrc), np.int8)
    nlo = np.zeros(nbins, np.int64)
    for b in range(nbins):
        s, e = starts[b], starts[b + 1]
        p = psrc[s:e]
        lofix = p < cfg.BBASE
        hifix = p >= cfg.SPLIT
        flex = ~lofix & ~hifix
        a, bb, f = int(lofix.sum()), int(hifix.sum()), int(flex.sum())
        x = int(np.clip((bb + f - a + 1) // 2, 0, f))
        g = np.zeros(e - s, np.int8)
        g[hifix] = 1
        fi = np.nonzero(flex)[0]
        g[fi[x:]] = 1
        grp[s:e] = g
        nlo[b] = a + x
    nhi = counts - nlo
    if cfg.NTAB <= cfg.SPLIT:
        grp[:] = 0
        nlo = counts.copy()
        nhi[:] = 0
    TLO = max(1, int(np.ceil(nlo.max() / 128)))
    THI = int(np.ceil(nhi.max() / 128))
    TT = TLO + THI

    idx_lo = np.zeros((NCORES, cfg.NSC, 128, C * TLO * 128 // 16), np.int16)
    idx_hi = np.zeros((NCORES, cfg.NSC, 128, max(1, C * THI * 128 // 16)),
                      np.int16)
    dstloc = np.full((NCORES, cfg.NSC, C * TT, 128), 255.0, np.float32)

    for core in range(NCORES):
        for sc in range(cfg.NSC):
            blocks = [core * NB + sc * C + j for j in range(C)]
            lo_list, hi_list = [], []
            for j, b in enumerate(blocks):
                s, e = starts[b], starts[b + 1]
                g = grp[s:e]
                p = psrc[s:e].copy()
                r = erow[s:e]
                for gi, (tbase, tcnt, lst) in enumerate(
                    ((0, TLO, lo_list), (C * TLO, THI, hi_list))
                ):
                    sel = g == gi
                    pp = p[sel]
                    rr = r[sel]
                    if gi == 1:
                        pp = pp - cfg.BBASE
                    assert len(pp) <= tcnt * 128
                    pad = tcnt * 128 - len(pp)
                    ppad = np.concatenate([pp, np.zeros(pad, np.int64)])
                    dpad = np.concatenate(
                        [rr.astype(np.float32),
                         np.full(pad, 255.0, np.float32)])
                    lst.append(ppad)
                    for t in range(tcnt):
                        gt = tbase + j * tcnt + t
                        dstloc[core, sc, gt] = dpad[t * 128:(t + 1) * 128]
            idx_lo[core, sc] = _wrap_idx16(np.concatenate(lo_list),
                                           C * TLO * 128)
            if THI:
                idx_hi[core, sc] = _wrap_idx16(np.concatenate(hi_list),
                                               C * THI * 128)

    # dstloc -> [128 edge-row, C*TT] per (core, sc)
    dstloc = np.ascontiguousarray(dstloc.transpose(0, 1, 3, 2))
    return dict(perm_pos=perm_pos, TLO=TLO, THI=THI,
                idx_lo=idx_lo, idx_hi=idx_hi, dstloc=dstloc.astype(BF))


def make_ablk(att_l):  # [H, HID] -> [FH, H]
    a = np.zeros((FH, H), np.float32)
    for h in range(H):
        a[h * HID:(h + 1) * HID, h] = att_l[h]
    return a


# ---------------------------------------------------------------- program


def build_program(cfg, TLO, THI, reps=1, ablate="none"):
    nc = bacc.Bacc("TRN2", target_bir_lowering=False, debug=False,
                   num_devices=NCORES, num_swdge_queues=cfg.NQ)
    NB, C, NSC, NPC, NTAB = cfg.NB, cfg.C, cfg.NSC, cfg.NPC, cfg.NTAB
    TT = TLO + THI
    CT = C * TT
    CE = CT * 128

    x_in = nc.dram_tensor("xp", [128, NPC], BF16, kind="ExternalInput")
    ilo = nc.dram_tensor("ilo", [NSC, 128, C * TLO * 128 // 16], I16,
                         kind="ExternalInput")
    ihi = nc.dram_tensor("ihi", [NSC, 128, max(1, C * THI * 128 // 16)], I16,
                         kind="ExternalInput")
    dstl = nc.dram_tensor("dstl", [NSC, 128, CT], BF16, kind="ExternalInput")
    wlr_in = nc.dram_tensor("wlr", [FH, L * 2 * FH], BF16,
                            kind="ExternalInput")
    ab_in = nc.dram_tensor("ab", [FH, L * H], BF16, kind="ExternalInput")
    blr_in = nc.dram_tensor("blr", [1, L * 2 * FH], F32, kind="ExternalInput")
    blrb_in = nc.dram_tensor("blrb", [128, L * 2 * FH], F32,
                             kind="ExternalInput")
    bo_in = nc.dram_tensor("bo", [128, L * FH], F32, kind="ExternalInput")
    wf_in = nc.dram_tensor("wfb", [128, FH], BF16, kind="ExternalInput")
    bf_in = nc.dram_tensor("bfb", [128, 1], F32, kind="ExternalInput")
    iota_in = nc.dram_tensor("iota", [128, 128], BF16, kind="ExternalInput")
    id_in = nc.dram_tensor("ident", [128, 128], BF16, kind="ExternalInput")
    out_t = nc.dram_tensor("out", [128, NB], F32, kind="ExternalOutput")

    with tile.TileContext(nc) as tc:
        with (
            tc.tile_pool(name="const", bufs=1) as cpool,
            tc.tile_pool(name="big", bufs=1) as bigp,
            tc.tile_pool(name="dram", bufs=1, space="DRAM") as dram,
        ):
            nc.gpsimd.load_library(mlp_lib)
            iota = cpool.tile([128, 128], BF16)
            nc.sync.dma_start(out=iota[:], in_=iota_in[:, :])
            ident = cpool.tile([128, 128], BF16)
            nc.sync.dma_start(out=ident[:], in_=id_in[:, :])
            wlr = cpool.tile([FH, L * 2 * FH], BF16)
            nc.sync.dma_start(out=wlr[:], in_=wlr_in[:, :])
            ab = cpool.tile([FH, L * H], BF16)
            nc.sync.dma_start(out=ab[:], in_=ab_in[:, :])
            blr = cpool.tile([1, L * 2 * FH], F32)
            nc.sync.dma_start(out=blr[:], in_=blr_in[:, :])
            blrb = cpool.tile([128, L * 2 * FH], F32)
            nc.sync.dma_start(out=blrb[:], in_=blrb_in[:, :])
            bo = cpool.tile([128, L * FH], F32)
            nc.sync.dma_start(out=bo[:], in_=bo_in[:, :])
            wfb = cpool.tile([128, FH], BF16)
            nc.sync.dma_start(out=wfb[:], in_=wf_in[:, :])
            bfb = cpool.tile([128, 1], F32)
            nc.sync.dma_start(out=bfb[:], in_=bf_in[:, :])
            ones1 = cpool.tile([1, 128], F32)
            nc.vector.memset(ones1[:], 1.0)

            hbuf = [bigp.tile([128, NPC], BF16, tag=f"h{i}", name=f"h{i}")
                    for i in range(2)]
            xr_pin = bigp.tile([128, NPC], BF16, tag="xrp")
            nc.gpsimd.dma_start(out=hbuf[0][:], in_=x_in[:, :])

            xl_shards = [dram.tile([NPC, FH], BF16, name=f"xl_shard{i}")
                         for i in range(L * reps)]
            xl_fulls = [dram.tile([NTAB, FH], BF16, addr_space="Shared",
                                  name=f"xl_full{i}") for i in range(L * reps)]
            h3f = bigp.tile([128, NPC], BF16, tag="h3f")

            for rep in range(reps):
              if rep > 0:
                nc.gpsimd.dma_start(out=hbuf[0][:], in_=x_in[:, :])
              for layer in range(L):
                h = hbuf[layer % 2]
                hn = hbuf[(layer + 1) % 2]
                last_layer = layer == L - 1
                xl_shard = xl_shards[rep * L + layer]
                xl_full = xl_fulls[rep * L + layer]
                # ------------- phase M (batched: 4 blocks per group)
                with (
                    tc.tile_pool(name=f"mp{layer}", bufs=2, space="PSUM") as mp,
                    tc.tile_pool(name=f"mx{layer}", bufs=2, space="PSUM") as mx,
                    tc.tile_pool(name=f"ms{layer}", bufs=3) as msb,
                ):
                    for b0 in range(0, NB, 4):
                        nbk = min(4, NB - b0)
                        hT_ps = mp.tile([128, 4, 128], BF16, tag="hT")
                        for j in range(nbk):
                            b = b0 + j
                            nc.tensor.transpose(
                                out=hT_ps[:, j, :],
                                in_=h[:, b * 128:(b + 1) * 128],
                                identity=ident[:])
                        hT = msb.tile([128, 4, 128], BF16, tag="hT")
                        nc.scalar.copy(hT[:, 0:nbk, :], hT_ps[:, 0:nbk, :])
                        ps = [mx.tile([128, 2, 256], F32, tag=f"x{i}",
                                      name=f"x{i}") for i in range(2)]
                        for j in range(nbk):
                            nc.tensor.matmul(
                                out=ps[j // 2][:, j % 2, :],
                                lhsT=hT[:, j, :],
                                rhs=wlr[:, layer * 2 * FH:
                                        (layer + 1) * 2 * FH],
                                start=True, stop=True)
                        xl_sb = msb.tile([128, 4, 128], BF16, tag="xlsb")
                        for i in range((nbk + 1) // 2):
                            nj = min(2, nbk - 2 * i)
                            nc.vector.tensor_add(
                                xl_sb[:, 2 * i:2 * i + nj, :],
                                ps[i][:, 0:nj, 0:128],
                                blrb[:, layer * 2 * FH:
                                     layer * 2 * FH + 128]
                                [:, None, :].to_broadcast([128, nj, 128]))
                            nc.vector.tensor_add(
                                xr_pin[:, (b0 + 2 * i) * 128:
                                       (b0 + 2 * i + nj) * 128]
                                .rearrange("p (j c) -> p j c", j=nj),
                                ps[i][:, 0:nj, 128:256],
                                blrb[:, layer * 2 * FH + 128:
                                     layer * 2 * FH + 256]
                                [:, None, :].to_broadcast([128, nj, 128]))
                        for j in range(nbk):
                            nc.sync.dma_start(
                                out=xl_shard[(b0 + j) * 128:
                                             (b0 + j + 1) * 128, :],
                                in_=xl_sb[:, j, :])

                nc.gpsimd.collective_compute(
                    "AllGather", OP.bypass,
                    replica_groups=[list(range(NCORES))],
                    ins=[xl_shard.opt()], outs=[xl_full.opt()],
                )

                # ------------- phase E
                with (
                    tc.tile_pool(name=f"ea{layer}", bufs=1, space="PSUM") as accp,
                    tc.tile_pool(name=f"ez{layer}", bufs=2, space="PSUM") as zp,
                    tc.tile_pool(name=f"eo{layer}", bufs=2, space="PSUM") as otp,
                    tc.tile_pool(name=f"el{layer}", bufs=2, space="PSUM") as lgp,
                    tc.tile_pool(name=f"es{layer}", bufs=3) as esb,
                    tc.tile_pool(name=f"eoh{layer}", bufs=2) as ohp,
                ):
                    for sc in range(NSC):
                        itlo = esb.tile([128, C * TLO * 128 // 16], I16,
                                        tag="itlo")
                        nc.sync.dma_start(out=itlo[:], in_=ilo[sc])
                        if THI:
                            ithi = esb.tile([128, C * THI * 128 // 16], I16,
                                            tag="ithi")
                            nc.sync.dma_start(out=ithi[:], in_=ihi[sc])
                        dl = esb.tile([128, CT], BF16, tag="dl")
                        nc.sync.dma_start(out=dl[:], in_=dstl[sc])

                        xln = esb.tile([128, CT, 128], BF16, tag="xln")
                        nlo_e = C * TLO * 128
                        if ablate == "nogather":
                            nc.vector.memset(xln[:, 0:1, :], 0.5)
                        else:
                            qb = 0
                            for base, ntile, it in (
                                (0, C * TLO, itlo),
                                (C * TLO, C * THI, ithi if THI else None),
                            ):
                                if not ntile:
                                    continue
                                srcap = (xl_full[:, :] if base == 0
                                         else xl_full[cfg.BBASE:, :])
                                half = ntile // 2
                                parts = ([(0, half), (half, ntile - half)]
                                         if half else [(0, ntile)])
                                for (o, n) in parts:
                                    ne = n * 128
                                    # idx slice: 8 columns per tile
                                    nc.gpsimd.dma_gather(
                                        out_ap=xln[:, base + o:base + o + n, :],
                                        in_ap=srcap,
                                        idxs_ap=it[:, o * 8:(o + n) * 8],
                                        num_idxs=ne, num_idxs_reg=ne,
                                        elem_size=FH,
                                        queue_num=(4 * sc + qb) % cfg.NQ,
                                        single_packet=False)
                                    qb += 1

                        if ablate == "nocompute":
                            nc.vector.memset(
                                hn[:, sc * C * 128:(sc + 1) * C * 128], 0.01)
                            if last_layer:
                                nc.vector.memset(
                                    h3f[:, sc * C * 128:(sc + 1) * C * 128],
                                    0.01)
                            continue

                        def blk_of(t):
                            if t < C * TLO:
                                j = t // TLO
                                first = (t % TLO) == 0
                                last = THI == 0 and (t % TLO) == TLO - 1
                            else:
                                j = (t - C * TLO) // THI
                                first = False
                                last = ((t - C * TLO) % THI) == THI - 1
                            return j, first, last

                        # one-hots (4 batched DVE builds) + transposes
                        OHB = 9
                        assert CT % OHB == 0
                        oh_all = ohp.tile([128, CT, 128], BF16, tag="oh")
                        for g in range(CT // OHB):
                            nc.vector.tensor_tensor(
                                out=oh_all[:, g * OHB:(g + 1) * OHB, :],
                                in0=iota[:][:, None, :].to_broadcast(
                                    [128, OHB, 128]),
                                in1=dl[:, g * OHB:(g + 1) * OHB, None]
                                .to_broadcast([128, OHB, 128]),
                                op=OP.is_equal)
                        ohs = [oh_all[:, t, :] for t in range(CT)]
                        n8 = math.ceil(CT / 8)
                        ohT_sb = esb.tile([128, CT, 128], BF16, tag="ohT")
                        for g in range(n8):
                            t0, t1 = g * 8, min(g * 8 + 8, CT)
                            ohT_ps = otp.tile([128, 8 * 128], BF16, tag="ohT")
                            for t in range(t0, t1):
                                nc.tensor.transpose(
                                    out=ohT_ps[:, (t - t0) * 128:
                                               (t - t0 + 1) * 128],
                                    in_=ohs[t], identity=ident[:])
                            nc.scalar.copy(
                                ohT_sb[:, t0:t1, :],
                                ohT_ps[:, :(t1 - t0) * 128])

                        # z_pre = xr_expand + xl^T ; leaky; z^T in SBUF
                        zT_sb = esb.tile([128, CT, 128], BF16, tag="zT")
                        n4 = math.ceil(CT / 4)
                        for g in range(n4):
                            t0, t1 = g * 4, min(g * 4 + 4, CT)
                            zpre = zp.tile([128, 4 * 128], F32, tag="zpre")
                            for t in range(t0, t1):
                                j, _, _ = blk_of(t)
                                bcol = (sc * C + j) * 128
                                o = (t - t0) * 128
                                nc.tensor.matmul(
                                    out=zpre[:, o:o + 128],
                                    lhsT=xr_pin[:, bcol:bcol + 128],
                                    rhs=ohT_sb[:, t, :],
                                    start=True, stop=False)
                                nc.tensor.matmul(
                                    out=zpre[:, o:o + 128],
                                    lhsT=xln[:, t, :], rhs=ident[:],
                                    start=False, stop=True)
                            nc.scalar.activation(
                                zT_sb[:, t0:t1, :],
                                zpre[:, :(t1 - t0) * 128],
                                AF.Prelu, alpha=0.2)

                        # logits; exp on the small [e, h] logits only
                        lgs = esb.tile([128, CT * 4], BF16, tag="lgs")
                        nsub = math.ceil(CT / 16)
                        for si in range(nsub):
                            t0, t1 = si * 16, min(si * 16 + 16, CT)
                            nt = t1 - t0
                            lg = lgp.tile([128, 64], F32, tag="lg")
                            for t in range(t0, t1):
                                nc.tensor.matmul(
                                    out=lg[:, (t - t0) * 4:(t - t0) * 4 + 4],
                                    lhsT=zT_sb[:, t, :],
                                    rhs=ab[:, layer * H:(layer + 1) * H],
                                    start=True, stop=True)
                            nc.scalar.activation(
                                lgs[:, t0 * 4:t0 * 4 + nt * 4],
                                lg[:, 0:nt * 4], AF.Exp)

                        # mw = xln * w (w broadcast 32-wide via 4D AP), w cols
                        mw = esb.tile([128, CT, 132], BF16, tag="mw")
                        nc.vector.tensor_mul(
                            mw[:, :, 0:128].rearrange(
                                "p t (h c) -> p t h c", h=H),
                            xln[:].rearrange("p t (h c) -> p t h c", h=H),
                            lgs[:].rearrange("p (t h) -> p t h", h=H)
                            [:, :, :, None].to_broadcast([128, CT, H, HID]))
                        nc.vector.tensor_copy(
                            mw[:, :, 128:132],
                            lgs[:].rearrange("p (t h) -> p t h", h=H))

                        accs = [accp.tile([128, 132], F32, tag=f"acc{j}",
                                          name=f"acc{j}")
                                for j in range(C)]
                        for t in range(CT):
                            j, first, last = blk_of(t)
                            nc.tensor.matmul(
                                out=accs[j][:], lhsT=ohs[t],
                                rhs=mw[:, t, :], start=first, stop=last)

                        # epilogue: normalize, +bo, ELU -> hn
                        asb = esb.tile([128, C, 132], F32, tag="asb")
                        for j in range(C):
                            nc.scalar.copy(asb[:, j, :], accs[j][:])
                        rec = esb.tile([128, C * 4], F32, tag="rec")
                        nc.vector.tensor_scalar_max(
                            rec[:].rearrange("p (j h) -> p j h", j=C),
                            asb[:, :, 128:132], 1e-16)
                        nc.vector.reciprocal(rec[:], rec[:])
                        u = esb.tile([128, C * 128], F32, tag="u")
                        nc.vector.tensor_mul(
                            u[:].rearrange("p (j h c) -> p j h c", j=C, h=H),
                            asb[:, :, 0:128].rearrange(
                                "p j (h c) -> p j h c", h=H),
                            rec[:].rearrange("p (j h) -> p j h", j=C)
                            [:, :, :, None].to_broadcast([128, C, H, HID]))
                        nc.vector.tensor_add(
                            u[:].rearrange("p (j f) -> p j f", j=C),
                            u[:].rearrange("p (j f) -> p j f", j=C),
                            bo[:, layer * FH:(layer + 1) * FH]
                            [:, None, :].to_broadcast([128, C, FH]))
                        # elu(u) = (exp(min(u,0)) - 1) + relu(u)
                        r = esb.tile([128, C * 128], F32, tag="r")
                        nc.scalar.activation(r[:], u[:], AF.Relu)
                        tmin = esb.tile([128, C * 128], F32, tag="tmin")
                        nc.vector.tensor_scalar_min(tmin[:], u[:], 0.0)
                        s_ = esb.tile([128, C * 128], F32, tag="s")
                        nc.scalar.activation(s_[:], tmin[:], AF.Exp)
                        hdst = h3f if last_layer else hn
                        nc.vector.scalar_tensor_tensor(
                            out=hdst[:, sc * C * 128:(sc + 1) * C * 128],
                            in0=s_[:], scalar=-1.0, in1=r[:],
                            op0=OP.add, op1=OP.add)

            # ------------- final linear (f32)
            with tc.tile_pool(name="fin", bufs=1) as fin:
                fm = fin.tile([128, NB, 128], F32)
                nc.vector.tensor_mul(
                    fm[:], h3f[:].rearrange("p (b f) -> p b f", b=NB),
                    wfb[:][:, None, :].to_broadcast([128, NB, FH]))
                of = fin.tile([128, NB], F32)
                nc.vector.tensor_reduce(
                    out=of[:], in_=fm[:], axis=mybir.AxisListType.X,
                    op=OP.add)
                nc.vector.tensor_scalar_add(of[:], of[:], bfb[:, 0:1])
                nc.sync.dma_start(out=out_t[:, :], in_=of[:])

    nc.compile()
    return nc


# ---------------------------------------------------------------- inputs


def _to_bf(x):
    return np.asarray(x, np.float32).astype(BF)


def make_inputs(cfg, pre, inputs):
    NB, NPC = cfg.NB, cfg.NPC
    x = np.asarray(inputs["x"], np.float32)
    xp_all = np.zeros((cfg.NTAB, IN), np.float32)
    xp_all[pre["perm_pos"]] = x
    W_l = np.stack([inputs["W_l0"], *[inputs["W_l"][i] for i in range(L - 1)]])
    W_r = np.stack([inputs["W_r0"], *[inputs["W_r"][i] for i in range(L - 1)]])
    att = np.stack([inputs["att0"], *[inputs["att"][i] for i in range(L - 1)]])
    b_l = np.stack([inputs["b_l0"], *[inputs["b_l"][i] for i in range(L - 1)]])
    b_r = np.stack([inputs["b_r0"], *[inputs["b_r"][i] for i in range(L - 1)]])
    bo = np.stack([inputs["bo0"], *[inputs["bo"][i] for i in range(L - 1)]])
    ablk = np.stack([make_ablk(att[l]) for l in range(L)])
    blr = np.stack([b_l, b_r], axis=1).astype(np.float32)
    bo_b = np.repeat(np.asarray(bo, np.float32)[:, None, :], 128, axis=1)
    wf = np.asarray(inputs["W_f"], np.float32)
    wfb = np.repeat(wf[:, 0][None, :], 128, axis=0)
    bfb = np.full((128, 1), float(np.asarray(inputs["b_f"]).ravel()[0]),
                  np.float32)
    iota = np.repeat(np.arange(128, dtype=np.float32)[None, :], 128, axis=0)
    ident = np.eye(128, dtype=np.float32)

    wlr_p = np.concatenate(
        [np.concatenate([W_l[l], W_r[l]], axis=1) for l in range(L)], axis=1)
    ab_p = np.concatenate([ablk[l] for l in range(L)], axis=1)   # [FH, L*H]
    blr_p = blr.reshape(1, -1).astype(np.float32)                # [1, L*2*FH]
    bo_p = np.concatenate([bo_b[l] for l in range(L)], axis=1)   # [128, L*FH]
    blrb_p = np.repeat(blr_p, 128, axis=0)
    shared = dict(
        wlr=_to_bf(wlr_p), ab=_to_bf(ab_p), blr=blr_p,
        blrb=blrb_p.astype(np.float32),
        bo=bo_p.astype(np.float32), wfb=_to_bf(wfb), bfb=bfb,
        iota=_to_bf(iota), ident=_to_bf(ident),
    )
    in_maps = []
    for c in range(NCORES):
        xp = xp_all[c * NPC:(c + 1) * NPC]
        xp_t = np.ascontiguousarray(
            xp.reshape(NB, 128, IN).transpose(1, 0, 2)).reshape(128, NB * IN)
        in_maps.append(dict(
            xp=_to_bf(xp_t),
            ilo=pre["idx_lo"][c], ihi=pre["idx_hi"][c],
            dstl=pre["dstloc"][c],
            **shared,
        ))
    return in_maps


def assemble_output(cfg, pre, out_maps):
    full = np.zeros((cfg.NTAB,), np.float32)
    for c in range(NCORES):
        o = np.asarray(out_maps[c]["out"], np.float32)  # [128, NB]
        full[c * cfg.NPC:(c + 1) * cfg.NPC] = o.T.reshape(-1)
    return full[pre["perm_pos"]][:, None].astype(np.float32)



# ---------------------------------------------------------------- runner


class CompiledSPMD:
    """Compile the bass module once; run it many times on n_cores devices."""

    def __init__(self, nc, n_cores):
        import jax
        from jax.sharding import Mesh, PartitionSpec
        from jax.experimental.shard_map import shard_map
        from concourse import bass2jax
        from concourse.bass2jax import _bass_exec_p, install_neuronx_cc_hook
        self._jax = jax
        install_neuronx_cc_hook()
        self.nc = nc
        self.n_cores = n_cores
        partition_name = (nc.partition_id_tensor.name
                          if nc.partition_id_tensor else None)
        in_names, out_names, out_avals, zero_outs = [], [], [], []
        for alloc in nc.m.functions[0].allocations:
            if not isinstance(alloc, mybir.MemoryLocationSet):
                continue
            name = alloc.memorylocations[0].name
            if alloc.kind == "ExternalInput":
                if name != partition_name and name != (
                        nc.dbg_addr.name if nc.dbg_addr else None):
                    in_names.append(name)
            elif alloc.kind == "ExternalOutput":
                out_names.append(name)
                shape = tuple(alloc.tensor_shape)
                dtype = mybir.dt.np(alloc.dtype)
                out_avals.append(jax.core.ShapedArray(shape, dtype))
                zero_outs.append(np.zeros(shape, dtype))
        self.in_names, self.out_names = in_names, out_names
        self.out_avals, self.zero_outs = out_avals, zero_outs
        n_params, n_outs = len(in_names), len(out_names)
        all_in = list(in_names) + list(out_names)
        if nc.dbg_addr is not None:
            all_in.append(nc.dbg_addr.name)
        if partition_name is not None:
            all_in.append(partition_name)
        dbg_name = nc.dbg_addr.name if nc.dbg_addr is not None else None

        def _body(*args):
            operands = list(args)
            if dbg_name is not None:
                operands.append(jax.numpy.zeros((1, 2), jax.numpy.uint32))
            if partition_name is not None:
                operands.append(bass2jax.partition_id_tensor())
            outs = _bass_exec_p.bind(
                *operands, out_avals=tuple(out_avals),
                in_names=tuple(all_in), out_names=tuple(out_names),
                lowering_input_output_aliases=(),
                sim_require_finite=True, sim_require_nnan=True, nc=nc)
            return tuple(outs)

        devices = jax.devices()[:n_cores]
        assert len(devices) == n_cores
        self._mesh = Mesh(np.asarray(devices), ("core",))
        in_specs = (PartitionSpec("core"),) * (n_params + n_outs)
        out_specs = (PartitionSpec("core"),) * n_outs
        self._P = PartitionSpec
        self._fn = jax.jit(
            shard_map(_body, mesh=self._mesh, in_specs=in_specs,
                      out_specs=out_specs, check_rep=False),
            keep_unused=True)

    def prepare_inputs(self, in_maps):
        jax = self._jax
        assert len(in_maps) == self.n_cores
        concat_in = [
            np.concatenate([np.asarray(in_maps[c][n])
                            for c in range(self.n_cores)], axis=0)
            for n in self.in_names]
        concat_zeros = [
            np.zeros((self.n_cores * z.shape[0], *z.shape[1:]), z.dtype)
            for z in self.zero_outs]
        sh = jax.sharding.NamedSharding(self._mesh, self._P("core"))
        args = [jax.device_put(a, sh) for a in concat_in + concat_zeros]
        jax.block_until_ready(args)
        return args

    def run_to_maps(self, args):
        jax = self._jax
        outs = jax.block_until_ready(self._fn(*args))
        return [
            {name: np.asarray(outs[i]).reshape(
                self.n_cores, *self.out_avals[i].shape)[c]
             for i, name in enumerate(self.out_names)}
            for c in range(self.n_cores)]

    def time_exec(self, args, iters=20, warmup=3):
        import time as _time
        jax = self._jax
        for _ in range(warmup):
            out = self._fn(*args)
        jax.block_until_ready(out)
        t0 = _time.perf_counter()
        outs = [self._fn(*args) for _ in range(iters)]
        jax.block_until_ready(outs)
        return (_time.perf_counter() - t0) / iters


_COMPILED = {}


def kernel(**inputs):
    cfg = FULL
    pre = preprocess(cfg, np.asarray(inputs["edge_index"]))
    key = (cfg.N, pre["TLO"], pre["THI"])
    if key not in _COMPILED:
        nc = build_program(cfg, pre["TLO"], pre["THI"])
        _COMPILED[key] = CompiledSPMD(nc, NCORES)
    comp = _COMPILED[key]
    in_maps = make_inputs(cfg, pre, inputs)
    args = comp.prepare_inputs(in_maps)
    out_maps = comp.run_to_maps(args)
    return assemble_output(cfg, pre, out_maps)

